# revision 2
# baseline (speedup 1.0000x reference)
"""Trainium2 Bass kernel for nn_MoRAPEForCausalLM (MoR expert-choice routing).

Self-contained. kernel(**inputs) -> np.ndarray [2, 2048, 32000] fp32.

Sharding (8 cores, SPMD single NEFF): tokens sharded (batch = core//4,
quarter = core%4); activations feature-major [D, T] in SBUF; K/V + routing
state exchanged via AllGather; device-side top-k (threshold bisection +
prefix-sum compaction + indirect DMA); lm_head vocab-sharded. Per-core
behavior via partition_id registers (dynamic DMA slices) + per-core small
inputs (attention-rank exp bias).

v2: all matmuls single-pass fp16 (weights + activation splits), full weight
packs passed per-core as inputs (no weight AllGather), resident [P,KD,1024]
weight tiles with 256KB+ DMAs, fused K+V collective per block, fp16 lm_head.
Routing logits/bisect/top-k stay exact fp32 on DVE.
"""
import math

import numpy as np

import concourse.bass as bass
import concourse.mybir as mybir
import concourse.tile as tile
from concourse import bacc
from concourse.bass import ts, ds
from concourse.bass_utils import run_bass_kernel_spmd
from concourse.expressions import smax
from concourse.masks import make_identity

P = 128
f32 = mybir.dt.float32
f32r = mybir.dt.float32r
f16 = mybir.dt.float16
i32 = mybir.dt.int32
AF = mybir.ActivationFunctionType
OP = mybir.AluOpType

B, S, D, H, DH, F, V = 2, 2048, 1024, 16, 64, 4096, 32000
R, NRANK = 8, 4
ALPHA, EPS = 0.1, 1e-6
KD, KF = D // P, F // P
T0 = B * S // R          # 512
T1 = T0 // 2             # 256
T2 = T0 // 4             # 128
VS = V // R              # 4000
ISQ = 1.0 / math.sqrt(DH)
E1 = DH + 1              # 65

BISECT_ITERS = 26
KGRP = 8
REPL = [list(range(R))]
NEG = -30.0

WSHAPES = {'wq': (D, D), 'wk': (D, D), 'wv': (D, D), 'wo': (D, D),
           'wg': (D, F), 'wu': (D, F), 'wd': (F, D)}
WNAMES = ('wq', 'wk', 'wv', 'wo', 'wg', 'wu', 'wd')
REFNAMES = {'wq': 'Wq', 'wk': 'Wk', 'wv': 'Wv', 'wo': 'Wo',
            'wg': 'Wg', 'wu': 'Wu', 'wd': 'Wd'}


def make_pack_meta():
    meta = {}
    for blk in range(6):
        items = []
        off = 0
        for wn in WNAMES:
            rows, cols = WSHAPES[wn]
            items.append((wn, rows, cols, off))
            off += (rows // R) * cols
        meta[blk] = (items, off)
    return meta


PACK_META = make_pack_meta()


class CX:
    pass


def load_wgroup(cx, pool, blk, wn, kbase, c0, cn, tag, nk=KD, bufs=2):
    """Resident [P, nk, cn] f16 weight tile; row-tiles kbase..kbase+nk,
    col slice [c0, c0+cn). One big DMA per row-tile."""
    nc = cx.nc
    items, _ = PACK_META[blk]
    for k, rows, cols, off in items:
        if k == wn:
            wt = pool.tile([P, nk, cn], f16, tag=tag, bufs=bufs,
                           name=f"{tag}_{cx.uid()}")
            rpr = rows // R
            for kk in range(nk):
                row0 = (kbase + kk) * P
                rank, rrow = row0 // rpr, row0 % rpr
                apv = cx.wpacks[blk][rank, ds(off + rrow * cols, P * cols)]
                apm = apv.rearrange("(p c) -> p c", c=cols)
                nc.sync.dma_start(wt[:, kk], apm[:, ds(c0, cn)])
            return wt
    raise KeyError(wn)


def split16(cx, pool, src_ap, tag, Tc, rows=P, bufs=1):
    nc = cx.nc
    hi = pool.tile([rows, Tc], f16, tag=f"{tag}h", bufs=bufs,
                   name=f"{tag}h_{cx.uid()}")
    nc.vector.tensor_copy(hi[:], src_ap)
    return hi


def rmsnorm(cx, pool, x_tiles, g_row, T, tag):
    nc = cx.nc
    sq = pool.tile([P, T], f32, tag="nsq", bufs=2, name=f"nsq_{cx.uid()}")
    ssum = cx.psC.tile([1, T], f32, tag="mis1", name=f"nss_{cx.uid()}")
    for ko in range(KD):
        nc.vector.tensor_mul(sq[:], x_tiles[ko][:], x_tiles[ko][:])
        nc.tensor.matmul(ssum[:], cx.ones_col[:], sq[:],
                         start=(ko == 0), stop=(ko == KD - 1))
    rms = pool.tile([1, T], f32, tag="nrm", bufs=1, name=f"nrm_{cx.uid()}")
    nc.vector.tensor_scalar(rms[:], ssum[:], 1.0 / D, EPS, op0=OP.mult, op1=OP.add)
    nc.scalar.activation(rms[:], rms[:], AF.Sqrt)
    rinv = pool.tile([1, T], f32, tag="nri", bufs=1, name=f"nri_{cx.uid()}")
    nc.vector.reciprocal(rinv[:], rms[:])
    bc = cx.psC.tile([P, T], f32, tag="mis2", name=f"nbc_{cx.uid()}")
    nc.tensor.matmul(bc[:], cx.ones_row[:], rinv[:], start=True, stop=True)
    bcs = pool.tile([P, T], f32, tag="nbcs", bufs=1, name=f"nbcs_{cx.uid()}")
    nc.vector.tensor_copy(bcs[:], bc[:])
    out = []
    for ko in range(KD):
        xn = pool.tile([P, T], f32, tag="nxn", bufs=2, name=f"nxn_{cx.uid()}")
        nc.vector.tensor_mul(xn[:], x_tiles[ko][:], bcs[:])
        nc.vector.tensor_tensor(
            xn[:, None, :], xn[:, None, :],
            cx.ln_sb[:, g_row, ko, None, None].to_broadcast([P, 1, T]), OP.mult)
        out.append(split16(cx, pool, xn[:], f"{tag}{ko}", T))
    return out


def linear_res(cx, wt, xin, T, Mtiles, out_cb):
    """out[m] = sum_ko wt[:,ko,m-slice].T @ xin[ko], Mtiles output tiles."""
    nc = cx.nc
    for mg in range(0, Mtiles, 2):
        pts = [cx.psA.tile([P, T], f32, tag=("ps" if mi == 0 else "sc"),
                           name=f"lps{mi}_{cx.uid()}") for mi in range(2)]
        for ko in range(KD):
            for mi in range(2):
                nc.tensor.matmul(pts[mi][:], wt[:, ko, ds((mg + mi) * P, P)],
                                 xin[ko][:], start=(ko == 0), stop=(ko == KD - 1))
        for mi in range(2):
            out_cb(mg + mi, pts[mi])


def llama_block(cx, dram, x_tiles, blk, T):
    nc, tc = cx.nc, cx.tc
    SK = T // P
    tg = f"b{blk}"
    KVL = D * T + T * H * E1

    with tc.tile_pool(name=f"bp{blk}", bufs=1) as bp:
        q_sp = [None] * KD
        kvloc = dram.tile([KVL], f16, tag=f"{tg}kv", name=f"{tg}kv")
        kloc = kvloc[ds(0, D * T)].rearrange("(d t) -> d t", t=T)
        vloc = kvloc[ds(D * T, T * H * E1)].rearrange("(t e) -> t e", e=H * E1)

        with tc.tile_pool(name=f"qk{blk}", bufs=2) as sp:
            xn = rmsnorm(cx, sp, x_tiles, 2 * blk, T, "xn")

            wqt = load_wgroup(cx, sp, blk, 'wq', 0, 0, D, "wqkv")

            def q_cb(mo, pt):
                q_sp[mo] = split16(cx, bp, pt[:], f"qs{mo}", T)

            linear_res(cx, wqt, xn, T, KD, q_cb)

            wkt = load_wgroup(cx, sp, blk, 'wk', 0, 0, D, "wqkv")

            def k_cb(mo, pt):
                kh = split16(cx, sp, pt[:], "kk", T, bufs=2)
                nc.sync.dma_start(kloc[ds(mo * P, P)], kh[:])

            linear_res(cx, wkt, xn, T, KD, k_cb)

            wvt = load_wgroup(cx, sp, blk, 'wv', 0, 0, D, "wqkv")
            for tt in range(SK):
                vsb = sp.tile([P, H * E1], f16, tag="vsb", bufs=2,
                              name=f"vsb_{cx.uid()}")
                nc.vector.memset(vsb[:], 1.0)
                for nc2 in range(D // 512):
                    pt = cx.psA.tile([P, 512], f32, tag="ps", name=f"vps_{cx.uid()}")
                    for ko in range(KD):
                        nc.tensor.matmul(pt[:], xn[ko][:, ts(tt, P)],
                                         wvt[:, ko, ds(nc2 * 512, 512)],
                                         start=(ko == 0), stop=(ko == KD - 1))
                    nh = 512 // DH
                    nc.vector.tensor_copy(
                        vsb[:, ds(nc2 * nh * E1, nh * E1)].rearrange(
                            "p (h e) -> p h e", e=E1)[:, :, :DH],
                        pt[:].rearrange("p (h e) -> p h e", e=DH))
                nc.sync.dma_start(vloc[ds(tt * P, P)], vsb[:])

        kvall = dram.tile([R, KVL], f16, tag=f"{tg}kva", name=f"{tg}kva",
                          addr_space="Shared")
        nc.gpsimd.collective_compute("AllGather", OP.bypass, replica_groups=REPL,
                                     ins=[kvloc[:].opt()], outs=[kvall[:].opt()])
        kall_r = kvall[:, ds(0, D * T)].rearrange("r (d t) -> (r d) t", t=T)
        vall_r = kvall[:, ds(D * T, T * H * E1)].rearrange(
            "r (t e) -> (r t) e", e=H * E1)

        attn_sp = [None] * KD
        with tc.tile_pool(name=f"at{blk}", bufs=2) as sp:
            vbufs = []
            for jrel in range(NRANK):
                src = cx.srcs[jrel]
                vb = sp.tile([P, SK, H * E1], f16, tag=f"vb{jrel}", bufs=1,
                             name=f"vb{jrel}_{cx.uid()}")
                for kk in range(SK):
                    nc.sync.dma_start(vb[:, kk],
                                      vall_r[ds(src * T + kk * P, P)])
                vbufs.append(vb)
            for hp in range(H // 2):
                kbufs = []
                for jrel in range(NRANK):
                    src = cx.srcs[jrel]
                    kb = sp.tile([P, T], f16, tag=f"kb{jrel}", bufs=2,
                                 name=f"kb{jrel}_{cx.uid()}")
                    nc.sync.dma_start(kb[:], kall_r[ds(src * D + hp * P, P)])
                    kbufs.append(kb)
                recip = sp.tile([33, T], f32, tag="rc", bufs=2, name=f"rc_{cx.uid()}")
                ovs = []
                for hpar in range(2):
                    h = 2 * hp + hpar
                    qrow = DH * hpar
                    rh = q_sp[hp][ds(qrow, DH)]
                    ov = cx.psB.tile([E1, T], f32, tag="ov",
                                     name=f"ov_{cx.uid()}")
                    total_sk = NRANK * SK
                    isk = 0
                    for jrel in range(NRANK):
                        for kk in range(SK):
                            sc = cx.psA.tile([P, T], f32, tag="sc",
                                             name=f"sc_{cx.uid()}")
                            nc.tensor.matmul(sc[:],
                                             kbufs[jrel][ds(qrow, DH), ts(kk, P)],
                                             rh, start=True, stop=True)
                            ex = sp.tile([P, T], f16, tag="ex", bufs=2,
                                         name=f"ex_{cx.uid()}")
                            if jrel == 0:
                                tmp = sp.tile([P, T], f32, tag="ext", bufs=2,
                                              name=f"ext_{cx.uid()}")
                                nc.scalar.activation(tmp[:], sc[:], AF.Exp, scale=ISQ)
                                nc.gpsimd.affine_select(
                                    ex[:], tmp[:], pattern=[[1, T]],
                                    compare_op=OP.is_ge, fill=0.0,
                                    base=-kk * P, channel_multiplier=-1)
                            else:
                                nc.scalar.activation(ex[:], sc[:], AF.Exp, scale=ISQ,
                                                     bias=cx.ab_sb[:, jrel:jrel + 1])
                            nc.tensor.matmul(ov[:],
                                             vbufs[jrel][:, kk, ds(h * E1, E1)],
                                             ex[:], start=(isk == 0),
                                             stop=(isk == total_sk - 1))
                            isk += 1
                    nc.vector.reciprocal(recip[ds(32 * hpar, 1)], ov[ds(DH, 1)])
                    ovs.append(ov)
                nbc = cx.psC.tile([P, T], f32, tag="mis2", name=f"nb_{cx.uid()}")
                nc.tensor.matmul(nbc[:], cx.sel2[:], recip[:], start=True, stop=True)
                nbs = sp.tile([P, T], f32, tag="nbs", bufs=2, name=f"nbs_{cx.uid()}")
                nc.vector.tensor_copy(nbs[:], nbc[:])
                at_f = sp.tile([P, T], f32, tag="atf", bufs=2, name=f"atf_{cx.uid()}")
                nc.vector.tensor_mul(at_f[ds(0, DH)], ovs[0][ds(0, DH)],
                                     nbs[ds(0, DH)])
                nc.vector.tensor_mul(at_f[ds(DH, DH)], ovs[1][ds(0, DH)],
                                     nbs[ds(DH, DH)])
                attn_sp[hp] = split16(cx, bp, at_f[:], f"as{hp}", T)

        with tc.tile_pool(name=f"op{blk}", bufs=2) as sp:
            wot = load_wgroup(cx, sp, blk, 'wo', 0, 0, D, "wot")

            def o_cb(mo, pt):
                nc.vector.tensor_add(x_tiles[mo][:], x_tiles[mo][:], pt[:])

            linear_res(cx, wot, attn_sp, T, KD, o_cb)

    with tc.tile_pool(name=f"ml{blk}", bufs=2) as sp:
        xn2 = rmsnorm(cx, sp, x_tiles, 2 * blk + 1, T, "xm")
        for g0 in range(0, KF, KGRP):
            wgt = load_wgroup(cx, sp, blk, 'wg', 0, g0 * P, KGRP * P, "wgt")
            wut = load_wgroup(cx, sp, blk, 'wu', 0, g0 * P, KGRP * P, "wut")
            gu_sp = [None] * KGRP
            for f0 in range(0, KGRP, 2):
                gps = [cx.psA.tile([P, T], f32, tag=t_, name=f"g{mi}_{cx.uid()}")
                       for mi, t_ in enumerate(("ps", "sc"))]
                ups = [cx.psB.tile([P, T], f32, tag="ov", name=f"u0_{cx.uid()}"),
                       cx.psC.tile([P, T], f32, tag="mis2", name=f"u1_{cx.uid()}")]
                for ko in range(KD):
                    xh = xn2[ko]
                    for mi in range(2):
                        nc.tensor.matmul(gps[mi][:],
                                         wgt[:, ko, ds((f0 + mi) * P, P)], xh[:],
                                         start=(ko == 0), stop=(ko == KD - 1))
                        nc.tensor.matmul(ups[mi][:],
                                         wut[:, ko, ds((f0 + mi) * P, P)], xh[:],
                                         start=(ko == 0), stop=(ko == KD - 1))
                for mi in range(2):
                    gs = sp.tile([P, T], f32, tag="gss", bufs=2,
                                 name=f"gss_{cx.uid()}")
                    nc.scalar.activation(gs[:], gps[mi][:], AF.Silu)
                    gu_f = sp.tile([P, T], f32, tag="guf", bufs=2,
                                   name=f"guf_{cx.uid()}")
                    nc.vector.tensor_mul(gu_f[:], gs[:], ups[mi][:])
                    gu_sp[f0 + mi] = split16(cx, sp, gu_f[:], f"gu{f0 + mi}", T)
            wdt = load_wgroup(cx, sp, blk, 'wd', g0, 0, D, "wdt", nk=KGRP)
            for mg in range(0, KD, 2):
                pts = [cx.psA.tile([P, T], f32, tag=t_, name=f"d{mi}_{cx.uid()}")
                       for mi, t_ in enumerate(("ps", "sc"))]
                for k2 in range(KGRP):
                    for mi in range(2):
                        nc.tensor.matmul(pts[mi][:],
                                         wdt[:, k2, ds((mg + mi) * P, P)],
                                         gu_sp[k2][:],
                                         start=(k2 == 0), stop=(k2 == KGRP - 1))
                for mi in range(2):
                    nc.vector.tensor_add(x_tiles[mg + mi][:],
                                         x_tiles[mg + mi][:], pts[mi][:])


def dve_matvec(cx, pool, x_tiles, rw_row, T):
    nc = cx.nc
    acc = pool.tile([P, T], f32, tag="mvac", bufs=1, name=f"mvac_{cx.uid()}")
    tmp = pool.tile([P, T], f32, tag="mvtp", bufs=1, name=f"mvtp_{cx.uid()}")
    for ko in range(KD):
        dst = acc if ko == 0 else tmp
        nc.vector.tensor_tensor(
            dst[:, None, :], x_tiles[ko][:, None, :],
            cx.rw_sb[:, rw_row, ko, None, None].to_broadcast([P, 1, T]), OP.mult)
        if ko > 0:
            nc.vector.tensor_add(acc[:], acc[:], tmp[:])
    pt = cx.psC.tile([1, T], f32, tag="mis1", name=f"mv_{cx.uid()}")
    nc.tensor.matmul(pt[:], cx.ones_col[:], acc[:], start=True, stop=True)
    lg = pool.tile([1, T], f32, tag="mvlg", bufs=1, name=f"mvlg_{cx.uid()}")
    nc.vector.tensor_copy(lg[:], pt[:])
    return lg


def bisect_mask(cx, pool, lall_flat, Sb, ktarget):
    nc = cx.nc
    nb = Sb // P
    lg = pool.tile([P, B, nb], f32, tag="bilg", bufs=1, name=f"bilg_{cx.uid()}")
    for bb in range(B):
        nc.sync.dma_start(lg[:, bb],
                          lall_flat[ds(bb * Sb, Sb)].rearrange("(p c) -> p c", c=nb))
    lo = pool.tile([P, B, nb], f32, tag="bilo", bufs=1, name=f"bilo_{cx.uid()}")
    hi = pool.tile([P, B, nb], f32, tag="bihi", bufs=1, name=f"bihi_{cx.uid()}")
    nc.vector.memset(lo[:], -16.0)
    nc.vector.memset(hi[:], 16.0)
    mid = pool.tile([P, B, nb], f32, tag="bimd", bufs=1, name=f"bimd_{cx.uid()}")
    cmp = pool.tile([P, B, nb], f32, tag="bicp", bufs=1, name=f"bicp_{cx.uid()}")
    red = pool.tile([P, B, 1], f32, tag="bird", bufs=1, name=f"bird_{cx.uid()}")
    cnt_sb = pool.tile([1, B], f32, tag="bict", bufs=1, name=f"bict_{cx.uid()}")
    pred = pool.tile([P, B], f32, tag="bipd", bufs=1, name=f"bipd_{cx.uid()}")
    dlt = pool.tile([P, B, nb], f32, tag="bidl", bufs=1, name=f"bidl_{cx.uid()}")
    for _ in range(BISECT_ITERS):
        nc.vector.tensor_add(mid[:], lo[:], hi[:])
        nc.vector.tensor_scalar_mul(mid[:], mid[:], 0.5)
        nc.vector.tensor_tensor(cmp[:], lg[:], mid[:], OP.is_gt)
        nc.vector.tensor_reduce(red[:], cmp[:], axis=mybir.AxisListType.X, op=OP.add)
        cnt = cx.psC.tile([1, B], f32, tag="mis1", name=f"bic_{cx.uid()}")
        nc.tensor.matmul(cnt[:], cx.ones_col[:], red[:, :, 0], start=True, stop=True)
        nc.vector.tensor_copy(cnt_sb[:], cnt[:])
        cbc = cx.psC.tile([P, B], f32, tag="mis2", name=f"bib_{cx.uid()}")
        nc.tensor.matmul(cbc[:], cx.ones_row[:], cnt_sb[:], start=True, stop=True)
        nc.vector.tensor_scalar(pred[:], cbc[:], float(ktarget), None, op0=OP.is_ge)
        # lo += pred * (mid - lo); hi += (1 - pred) * (mid - hi)
        nc.vector.tensor_sub(dlt[:], mid[:], lo[:])
        nc.vector.tensor_tensor(dlt[:], dlt[:],
                                pred[:, :, None].to_broadcast([P, B, nb]), OP.mult)
        nc.vector.tensor_add(lo[:], lo[:], dlt[:])
        nc.vector.tensor_scalar(pred[:], cbc[:], float(ktarget), None, op0=OP.is_lt)
        nc.vector.tensor_sub(dlt[:], mid[:], hi[:])
        nc.vector.tensor_tensor(dlt[:], dlt[:],
                                pred[:, :, None].to_broadcast([P, B, nb]), OP.mult)
        nc.vector.tensor_add(hi[:], hi[:], dlt[:])
    mask = pool.tile([P, B, nb], f32, tag="bimk", bufs=1, name=f"bimk_{cx.uid()}")
    nc.vector.tensor_tensor(mask[:], lg[:], lo[:], OP.is_gt)
    return mask


def cumsum_pos(cx, pool, dram, mask, Sb, ksel, tag):
    nc = cx.nc
    nb = Sb // P
    a = mask
    s, pp = 1, 0
    while s < nb:
        bt = pool.tile([P, B, nb], f32, tag=f"cs{pp % 2}", bufs=1,
                       name=f"cs_{cx.uid()}")
        nc.vector.tensor_copy(bt[:, :, :s], a[:, :, :s])
        nc.vector.tensor_add(bt[:, :, s:], a[:, :, s:], a[:, :, :nb - s])
        a = bt
        s *= 2
        pp += 1
    tot = pool.tile([P, B], f32, tag="cstt", bufs=1, name=f"cstt_{cx.uid()}")
    nc.vector.tensor_copy(tot[:], a[:, :, nb - 1])
    ppf = cx.psC.tile([P, B], f32, tag="mis2", name=f"csp_{cx.uid()}")
    nc.tensor.matmul(ppf[:], cx.triu[:], tot[:], start=True, stop=True)
    cs = pool.tile([P, B, nb], f32, tag="cscs", bufs=1, name=f"cscs_{cx.uid()}")
    nc.vector.tensor_tensor(cs[:], a[:], ppf[:, :, None].to_broadcast([P, B, nb]),
                            OP.add)
    csd = dram.tile([B, Sb], f32, tag=f"{tag}csd", name=f"{tag}csd")
    nc.sync.dma_start(csd[:].rearrange("b (p c) -> p b c", p=P), cs[:])
    posd = dram.tile([B * ksel, 1], f32, tag=f"{tag}posd", name=f"{tag}posd")
    for bb in range(B):
        csrow = pool.tile([1, Sb], f32, tag="cscr", bufs=1, name=f"cscr_{cx.uid()}")
        nc.sync.dma_start(csrow[:], csd[bb, None, :])
        cbc = pool.tile([P, Sb], f32, tag="cscb", bufs=1, name=f"cscb_{cx.uid()}")
        for ch in range(0, Sb, 512):
            w = min(512, Sb - ch)
            pt = cx.psC.tile([P, 512], f32, tag="mis2", name=f"csq_{cx.uid()}")
            nc.tensor.matmul(pt[:, :w], cx.ones_row[:], csrow[:, ds(ch, w)],
                             start=True, stop=True)
            nc.vector.tensor_copy(cbc[:, ds(ch, w)], pt[:, :w])
        for rt in range(ksel // P):
            rcol = pool.tile([P, 1], f32, tag="csrc", bufs=2, name=f"csrc_{cx.uid()}")
            nc.vector.tensor_scalar_add(rcol[:], cx.iota_f[:], float(rt * P))
            cmp = pool.tile([P, Sb], f32, tag="cscm", bufs=2, name=f"cscm_{cx.uid()}")
            nc.vector.tensor_tensor(cmp[:], cbc[:], rcol[:].to_broadcast([P, Sb]),
                                    OP.is_le)
            red = pool.tile([P, 1], f32, tag="csrd", bufs=2, name=f"csrd_{cx.uid()}")
            nc.vector.tensor_reduce(red[:], cmp[:], axis=mybir.AxisListType.X,
                                    op=OP.add)
            nc.sync.dma_start(posd[ds(bb * ksel + rt * P, P)], red[:])
    return posd


def to_tok_dram(cx, pool, dtile, x_tiles, T):
    nc = cx.nc
    for tt in range(T // P):
        asm = pool.tile([P, D], f32, tag="tkas", bufs=2, name=f"tkas_{cx.uid()}")
        for ko in range(KD):
            tr = cx.psC.tile([P, P], f32, tag="mis2", name=f"tktr_{cx.uid()}")
            nc.tensor.transpose(tr[:], x_tiles[ko][:, ts(tt, P)], cx.ident[:])
            nc.vector.tensor_copy(asm[:, ts(ko, P)], tr[:])
        nc.sync.dma_start(dtile[ds(tt * P, P)], asm[:])


def gather_sel(cx, pool, res, src_flat, posd, T, boff_col, rtag):
    nc = cx.nc
    myoff = cx.pid * T
    xt = [res.tile([P, T], f32, tag=f"{rtag}{ko}", name=f"{rtag}{ko}")
          for ko in range(KD)]
    for u in range(T // P):
        pv = pool.tile([P, 1], f32, tag="gspv", bufs=2, name=f"gspv_{cx.uid()}")
        nc.sync.dma_start(pv[:], posd[ds(myoff + u * P, P)])
        nc.vector.tensor_scalar(pv[:], pv[:], boff_col, None, op0=OP.add)
        pi = pool.tile([P, 1], i32, tag="gspi", bufs=2, name=f"gspi_{cx.uid()}")
        nc.vector.tensor_copy(pi[:], pv[:])
        g = pool.tile([P, D], f32, tag="gsg", bufs=2, name=f"gsg_{cx.uid()}")
        nc.gpsimd.indirect_dma_start(
            out=g[:], out_offset=None, in_=src_flat,
            in_offset=bass.IndirectOffsetOnAxis(ap=pi[:, :1], axis=0))
        for ko in range(KD):
            tr = cx.psC.tile([P, P], f32, tag="mis2", name=f"gstr_{cx.uid()}")
            nc.tensor.transpose(tr[:], g[:, ts(ko, P)], cx.ident[:])
            nc.vector.tensor_copy(xt[ko][:, ts(u, P)], tr[:])
    return xt


def topw_bcast(cx, pool, sel_in, rw_row, T):
    nc = cx.nc
    lgs = dve_matvec(cx, pool, sel_in, rw_row, T)
    tw = pool.tile([1, T], f32, tag="twr", bufs=1, name=f"twr_{cx.uid()}")
    nc.scalar.activation(tw[:], lgs[:], AF.Sigmoid)
    nc.vector.tensor_scalar_mul(tw[:], tw[:], ALPHA)
    pt = cx.psC.tile([P, T], f32, tag="mis2", name=f"twp_{cx.uid()}")
    nc.tensor.matmul(pt[:], cx.ones_row[:], tw[:], start=True, stop=True)
    twb = pool.tile([P, T], f32, tag="twb", bufs=1, name=f"twb_{cx.uid()}")
    nc.vector.tensor_copy(twb[:], pt[:])
    return twb


def build_program(stages=4, dbg=False):
    nc = bacc.Bacc("TRN2", target_bir_lowering=False)
    cx = CX()
    cx.nc = nc
    cx._u = 0

    def uid():
        cx._u += 1
        return cx._u
    cx.uid = uid

    innames = ["h0T", "ln", "rw", "abias", "fvec", "sel2c"]
    h0T = nc.declare_dram_parameter("h0T", [D, T0], f32, isOutput=False)
    lnp = nc.declare_dram_parameter("ln", [13, D], f32, isOutput=False)
    rwp = nc.declare_dram_parameter("rw", [2, D], f32, isOutput=False)
    abp = nc.declare_dram_parameter("abias", [NRANK, P], f32, isOutput=False)
    fvp = nc.declare_dram_parameter("fvec", [P, 4], f32, isOutput=False)
    s2p = nc.declare_dram_parameter("sel2c", [33, P], f32, isOutput=False)
    nblk = 6 if stages >= 3 else (3 if stages >= 2 else 1)
    wparams = {}
    for blk in range(nblk):
        items, shard = PACK_META[blk]
        wparams[blk] = nc.declare_dram_parameter(f"wpack{blk}", [R, shard], f16,
                                                 isOutput=False)
        innames.append(f"wpack{blk}")
    out = embT = None
    if stages >= 4:
        embT = nc.declare_dram_parameter("embT", [D, VS], f16, isOutput=False)
        out = nc.declare_dram_parameter("out", [B * S, VS], f32, isOutput=True)
        innames.append("embT")
    dbg_o = {}

    def dbg_out(nm, shp):
        dbg_o[nm] = nc.declare_dram_parameter(nm, shp, f32, isOutput=True)
        return dbg_o[nm]

    with tile.TileContext(nc) as tc:
        cx.tc = tc
        with (
            tc.tile_pool(name="const", bufs=1) as cst,
            tc.tile_pool(name="res", bufs=1) as res,
            tc.tile_pool(name="psA", bufs=2, space="PSUM") as psA,
            tc.tile_pool(name="psB", bufs=2, space="PSUM") as psB,
            tc.tile_pool(name="psC", bufs=1, space="PSUM") as psC,
            tc.tile_pool(name="dram", bufs=1, space="DRAM") as dram,
        ):
            cx.psA, cx.psB, cx.psC = psA, psB, psC

            cx.ones_col = cst.tile([P, 1], f32, name="ones_col")
            nc.vector.memset(cx.ones_col[:], 1.0)
            cx.ones_row = cst.tile([1, P], f32, name="ones_row")
            nc.vector.memset(cx.ones_row[:], 1.0)
            cx.sel2 = cst.tile([33, P], f32, name="sel2")
            nc.sync.dma_start(cx.sel2[:], s2p.ap())
            cx.ident = cst.tile([P, P], f32, name="ident")
            make_identity(nc, cx.ident[:])
            onespp = cst.tile([P, P], f32, name="onespp")
            nc.vector.memset(onespp[:], 1.0)
            cx.triu = cst.tile([P, P], f32, name="triu")
            nc.gpsimd.affine_select(cx.triu[:], onespp[:], pattern=[[1, P]],
                                    compare_op=OP.is_ge, fill=0.0, base=-1,
                                    channel_multiplier=-1)
            iota_i = cst.tile([P, 1], i32, name="iota_i")
            nc.gpsimd.iota(iota_i[:], pattern=[[0, 1]], base=0, channel_multiplier=1)
            cx.iota_f = cst.tile([P, 1], f32, name="iota_f")
            nc.vector.tensor_copy(cx.iota_f[:], iota_i[:])
            cx.ln_sb = cst.tile([P, 13, KD], f32, name="ln_sb")
            nc.sync.dma_start(cx.ln_sb[:],
                              lnp.ap().rearrange("r (ko p) -> p r ko", p=P))
            cx.rw_sb = cst.tile([P, 2, KD], f32, name="rw_sb")
            nc.sync.dma_start(cx.rw_sb[:],
                              rwp.ap().rearrange("r (ko p) -> p r ko", p=P))
            cx.ab_sb = cst.tile([P, NRANK], f32, name="ab_sb")
            nc.sync.dma_start(cx.ab_sb[:], abp.ap().rearrange("j p -> p j"))
            cx.fv_sb = cst.tile([P, 4], f32, name="fv_sb")
            nc.sync.dma_start(cx.fv_sb[:], fvp.ap())

            pid = nc.sync.partition_id()
            cx.pid = pid
            qreg = pid % NRANK
            base = pid - qreg
            cx.srcs = [smax(pid - j, base) for j in range(NRANK)]

            cx.wpacks = {}
            for blk in range(nblk):
                cx.wpacks[blk] = wparams[blk].ap()

            # ---- stage 1: block 0 + recursion-0 routing
            with tc.tile_pool(name="st1", bufs=1) as st1:
                x = [st1.tile([P, T0], f32, tag=f"xa{ko}", name=f"xa{ko}")
                     for ko in range(KD)]
                h0ap = h0T.ap().rearrange("(ko p) t -> p ko t", p=P)
                for ko in range(KD):
                    nc.sync.dma_start(x[ko][:], h0ap[:, ko])
                llama_block(cx, dram, x, 0, T0)

                with tc.tile_pool(name="rt0", bufs=2) as rp:
                    lg0 = dve_matvec(cx, rp, x, 0, T0)
                    lloc = dram.tile([1, T0], f32, tag="lloc0", name="lloc0")
                    nc.sync.dma_start(lloc[:], lg0[:])
                    lall = dram.tile([R, 1, T0], f32, tag="lall0", name="lall0",
                                     addr_space="Shared")
                    nc.gpsimd.collective_compute(
                        "AllGather", OP.bypass, replica_groups=REPL,
                        ins=[lloc[:].opt()], outs=[lall[:].opt()])
                    htl = dram.tile([T0, D], f32, tag="htl", name="htl")
                    to_tok_dram(cx, rp, htl, x, T0)
                    hta = dram.tile([R, T0, D], f32, tag="hta", name="hta",
                                    addr_space="Shared")
                    nc.gpsimd.collective_compute(
                        "AllGather", OP.bypass, replica_groups=REPL,
                        ins=[htl[:].opt()], outs=[hta[:].opt()])
                    cx.hta_r = hta[:].rearrange("r t d -> (r t) d")

                    mask0 = bisect_mask(cx, rp,
                                        lall[:].rearrange("r o t -> (r o t)"),
                                        S, S // 2)
                    posd0 = cumsum_pos(cx, rp, dram, mask0, S, S // 2, "c0")
                    seli = gather_sel(cx, rp, res, cx.hta_r, posd0, T1,
                                      cx.fv_sb[:, 0:1], "sli")
                    if dbg:
                        d1 = dbg_out("dbg_h0b", [T0, D])
                        nc.sync.dma_start(d1.ap(), htl[:])
                        d2 = dbg_out("dbg_lg", [1, T0])
                        nc.sync.dma_start(d2.ap(), lloc[:])
                        d3 = dbg_out("dbg_pos", [B * S // 2, 1])
                        nc.sync.dma_start(d3.ap(), posd0[:])
                        d4 = dbg_out("dbg_selT", [D, T1])
                        d4r = d4.ap().rearrange("(ko p) t -> p ko t", p=P)
                        for ko in range(KD):
                            nc.sync.dma_start(d4r[:, ko], seli[ko][:])

            if stages >= 2:
                with tc.tile_pool(name="st2", bufs=1) as st2:
                    sel = [st2.tile([P, T1], f32, tag=f"sl{ko}", name=f"sl{ko}")
                           for ko in range(KD)]
                    for ko in range(KD):
                        nc.vector.tensor_copy(sel[ko][:], seli[ko][:])
                    llama_block(cx, dram, sel, 1, T1)
                    llama_block(cx, dram, sel, 2, T1)
                    with tc.tile_pool(name="rt1", bufs=2) as rp:
                        twb0 = topw_bcast(cx, rp, seli, 0, T1)
                        x1 = [res.tile([P, T1], f32, tag=f"x1{ko}", name=f"x1{ko}")
                              for ko in range(KD)]
                        for ko in range(KD):
                            nc.vector.tensor_mul(x1[ko][:], sel[ko][:], twb0[:])
                            nc.vector.tensor_add(x1[ko][:], x1[ko][:], seli[ko][:])
                        lg1 = dve_matvec(cx, rp, x1, 1, T1)
                        lloc1 = dram.tile([1, T1], f32, tag="lloc1", name="lloc1")
                        nc.sync.dma_start(lloc1[:], lg1[:])
                        lall1 = dram.tile([R, 1, T1], f32, tag="lall1",
                                          name="lall1", addr_space="Shared")
                        nc.gpsimd.collective_compute(
                            "AllGather", OP.bypass, replica_groups=REPL,
                            ins=[lloc1[:].opt()], outs=[lall1[:].opt()])
                        x1l = dram.tile([T1, D], f32, tag="x1l", name="x1l")
                        to_tok_dram(cx, rp, x1l, x1, T1)
                        x1a = dram.tile([R, T1, D], f32, tag="x1a", name="x1a",
                                        addr_space="Shared")
                        nc.gpsimd.collective_compute(
                            "AllGather", OP.bypass, replica_groups=REPL,
                            ins=[x1l[:].opt()], outs=[x1a[:].opt()])
                        cx.x1a_r = x1a[:].rearrange("r t d -> (r t) d")

                        mask1 = bisect_mask(cx, rp,
                                            lall1[:].rearrange("r o t -> (r o t)"),
                                            S // 2, S // 4)
                        posd1 = cumsum_pos(cx, rp, dram, mask1, S // 2, S // 4, "c1")
                        sl1i = gather_sel(cx, rp, res, cx.x1a_r, posd1, T2,
                                          cx.fv_sb[:, 1:2], "s1i")
                        if dbg:
                            d5 = dbg_out("dbg_x1", [T1, D])
                            nc.sync.dma_start(d5.ap(), x1l[:])
                            d6 = dbg_out("dbg_pos1", [B * S // 4, 1])
                            nc.sync.dma_start(d6.ap(), posd1[:])

            if stages >= 3:
                with tc.tile_pool(name="st3", bufs=1) as st3:
                    sl1 = [st3.tile([P, T2], f32, tag=f"sm{ko}", name=f"sm{ko}")
                           for ko in range(KD)]
                    for ko in range(KD):
                        nc.vector.tensor_copy(sl1[ko][:], sl1i[ko][:])
                    llama_block(cx, dram, sl1, 3, T2)
                    llama_block(cx, dram, sl1, 4, T2)
                    with tc.tile_pool(name="rt2", bufs=2) as rp:
                        twb1 = topw_bcast(cx, rp, sl1i, 1, T2)
                        z = [st3.tile([P, T2], f32, tag=f"zz{ko}", name=f"zz{ko}")
                             for ko in range(KD)]
                        for ko in range(KD):
                            nc.vector.tensor_mul(z[ko][:], sl1[ko][:], twb1[:])
                            nc.vector.tensor_add(z[ko][:], z[ko][:], sl1i[ko][:])
                        zl = dram.tile([T2, D], f32, tag="zl", name="zl")
                        to_tok_dram(cx, rp, zl, z, T2)
                        za = dram.tile([R, T2, D], f32, tag="za", name="za",
                                       addr_space="Shared")
                        nc.gpsimd.collective_compute(
                            "AllGather", OP.bypass, replica_groups=REPL,
                            ins=[zl[:].opt()], outs=[za[:].opt()])
                        za_r = za[:].rearrange("r t d -> (r t) d")

                        h2loc = dram.tile([R * T0, D], f32, tag="h2loc",
                                          name="h2loc")
                        nc.sync.dma_start(h2loc[:], cx.hta_r)
                        cx.h2_r = h2loc[:]

                        for ch in range(B * S // 2 // P):
                            bb = ch // (S // 2 // P)
                            ssb = rp.tile([P, D], f32, tag="scx", bufs=2,
                                          name=f"scx_{cx.uid()}")
                            nc.sync.dma_start(ssb[:], cx.x1a_r[ds(ch * P, P)])
                            pv = rp.tile([P, 1], f32, tag="scp", bufs=2,
                                         name=f"scp_{cx.uid()}")
                            nc.sync.dma_start(pv[:], posd0[ds(ch * P, P)])
                            nc.vector.tensor_scalar_add(pv[:], pv[:], float(bb * S))
                            pi = rp.tile([P, 1], i32, tag="sci", bufs=2,
                                         name=f"sci_{cx.uid()}")
                            nc.vector.tensor_copy(pi[:], pv[:])
                            nc.gpsimd.indirect_dma_start(
                                out=cx.h2_r, out_offset=bass.IndirectOffsetOnAxis(
                                    ap=pi[:, :1], axis=0),
                                in_=ssb[:], in_offset=None)
                        for ch in range(B * S // 4 // P):
                            bb = ch // (S // 4 // P)
                            ssb = rp.tile([P, D], f32, tag="scz", bufs=2,
                                          name=f"scz_{cx.uid()}")
                            nc.sync.dma_start(ssb[:], za_r[ds(ch * P, P)])
                            p1 = rp.tile([P, 1], f32, tag="sc1", bufs=2,
                                         name=f"sc1_{cx.uid()}")
                            nc.sync.dma_start(p1[:], posd1[ds(ch * P, P)])
                            nc.vector.tensor_scalar_add(p1[:], p1[:],
                                                        float(bb * (S // 2)))
                            p1i = rp.tile([P, 1], i32, tag="sc2", bufs=2,
                                          name=f"sc2_{cx.uid()}")
                            nc.vector.tensor_copy(p1i[:], p1[:])
                            p0 = rp.tile([P, 1], f32, tag="sc3", bufs=2,
                                         name=f"sc3_{cx.uid()}")
                            nc.gpsimd.indirect_dma_start(
                                out=p0[:], out_offset=None, in_=posd0[:],
                                in_offset=bass.IndirectOffsetOnAxis(
                                    ap=p1i[:, :1], axis=0))
                            nc.vector.tensor_scalar_add(p0[:], p0[:], float(bb * S))
                            p0i = rp.tile([P, 1], i32, tag="sc4", bufs=2,
                                          name=f"sc4_{cx.uid()}")
                            nc.vector.tensor_copy(p0i[:], p0[:])
                            nc.gpsimd.indirect_dma_start(
                                out=cx.h2_r, out_offset=bass.IndirectOffsetOnAxis(
                                    ap=p0i[:, :1], axis=0),
                                in_=ssb[:], in_offset=None)
                        if dbg:
                            d7 = dbg_out("dbg_h2", [T0, D])
                            nc.sync.dma_start(d7.ap(), cx.h2_r[ds(cx.pid * T0, T0)])

            if stages >= 4:
                with tc.tile_pool(name="st4", bufs=1) as st4:
                    x5 = [st4.tile([P, T0], f32, tag=f"x5{ko}", name=f"x5{ko}")
                          for ko in range(KD)]
                    with tc.tile_pool(name="ld5", bufs=2) as rp:
                        for tt in range(T0 // P):
                            tkb = rp.tile([P, D], f32, tag="h2t", bufs=2,
                                          name=f"h2t_{cx.uid()}")
                            nc.sync.dma_start(tkb[:],
                                              cx.h2_r[ds(cx.pid * T0 + tt * P, P)])
                            for ko in range(KD):
                                tr = cx.psC.tile([P, P], f32, tag="mis2",
                                                 name=f"h2r_{cx.uid()}")
                                nc.tensor.transpose(tr[:], tkb[:, ts(ko, P)],
                                                    cx.ident[:])
                                nc.vector.tensor_copy(x5[ko][:, ts(tt, P)], tr[:])
                    llama_block(cx, dram, x5, 5, T0)
                    hfl = dram.tile([P, KD, T0], f16, tag="hfl", name="hfl")
                    with tc.tile_pool(name="fn5", bufs=2) as rp:
                        hfn = rmsnorm(cx, rp, x5, 12, T0, "hf")
                        for ko in range(KD):
                            nc.sync.dma_start(hfl[:, ko], hfn[ko][:])
                    hfa = dram.tile([R, P, KD, T0], f16, tag="hfa", name="hfa",
                                    addr_space="Shared")
                    nc.gpsimd.collective_compute(
                        "AllGather", OP.bypass, replica_groups=REPL,
                        ins=[hfl[:].opt()], outs=[hfa[:].opt()])
                with tc.tile_pool(name="hd", bufs=1) as hd:
                    NCH = 4
                    CH = VS // NCH  # 1000
                    for ch in range(NCH):
                        et = hd.tile([P, KD, CH], f16, tag="et", bufs=2,
                                     name=f"et_{cx.uid()}")
                        for ko in range(KD):
                            nc.sync.dma_start(
                                et[:, ko],
                                embT.ap()[ds(ko * P, P), ds(ch * CH, CH)])
                        for rr in range(R):
                            hl = hd.tile([P, KD, T0], f16, tag="hl", bufs=2,
                                         name=f"hl_{cx.uid()}")
                            nc.sync.dma_start(hl[:], hfa[rr])
                            for tt in range(T0 // P):
                                for hf2 in range(2):
                                    pt = cx.psA.tile([P, 500], f32, tag="ps",
                                                     name=f"hd_{cx.uid()}")
                                    for ko in range(KD):
                                        nc.tensor.matmul(
                                            pt[:], hl[:, ko, ts(tt, P)],
                                            et[:, ko, ds(hf2 * 500, 500)],
                                            start=(ko == 0), stop=(ko == KD - 1))
                                    ot = hd.tile([P, 500], f32, tag="hot", bufs=3,
                                                 name=f"hot_{cx.uid()}")
                                    nc.vector.tensor_copy(ot[:], pt[:])
                                    nc.sync.dma_start(
                                        out.ap()[ds(rr * T0 + tt * P, P),
                                                 ds(ch * CH + hf2 * 500, 500)],
                                        ot[:])
    nc.finalize()
    return nc, innames, list(dbg_o)


# ----------------------------------------------------------------------- host

_CACHE = {}


def _prepare_inmaps(inputs, stages):
    input_ids = np.asarray(inputs['input_ids'])
    embed = np.asarray(inputs['embed'], dtype=np.float32)
    pos_emb = np.asarray(inputs['pos_emb'], dtype=np.float32)
    h0 = embed[input_ids] + pos_emb[None, :, :]
    ln = np.empty((13, D), np.float32)
    for i in range(6):
        ln[2 * i] = inputs['ln1'][i]
        ln[2 * i + 1] = inputs['ln2'][i]
    ln[12] = inputs['final_norm']
    rw = np.asarray(inputs['router_w'], dtype=np.float32)

    nblk = 6 if stages >= 3 else (3 if stages >= 2 else 1)
    packs = {}
    for blk in range(nblk):
        items, shard = PACK_META[blk]
        pk = np.empty((R, shard), np.float16)
        for key, rows, cols, off in items:
            W = np.asarray(inputs[REFNAMES[key]][blk], dtype=np.float32)
            rpr = rows // R
            n = rpr * cols
            Wm = W.astype(np.float16).reshape(R, n)
            pk[:, off:off + n] = Wm
        packs[blk] = pk

    embT16 = None
    in_maps = []
    for c in range(R):
        b, q = c // NRANK, c % NRANK
        m = {}
        sl = h0[b, q * T0:(q + 1) * T0]
        m['h0T'] = np.ascontiguousarray(sl.T)
        m['ln'] = ln
        m['rw'] = rw
        ab = np.zeros((NRANK, P), np.float32)
        for j in range(NRANK):
            if j > q:
                ab[j] = NEG
        m['abias'] = ab
        m['fvec'] = np.tile(np.array([[b * S, b * (S // 2), 0, 0]], np.float32),
                            (P, 1))
        s2 = np.zeros((33, P), np.float32)
        s2[0, :DH] = 1.0
        s2[32, DH:] = 1.0
        m['sel2c'] = s2
        for blk in range(nblk):
            m[f'wpack{blk}'] = packs[blk]
        if stages >= 4:
            m['embT'] = np.ascontiguousarray(
                embed[c * VS:(c + 1) * VS].T.astype(np.float16))
        in_maps.append(m)
    return in_maps


def run(inputs, stages=4, dbg=False, trace=False):
    key = (stages, dbg)
    if key not in _CACHE:
        _CACHE[key] = build_program(stages, dbg)
    nc, innames, dbgnames = _CACHE[key]
    in_maps = _prepare_inmaps(inputs, stages)
    return run_bass_kernel_spmd(nc, in_maps, core_ids=list(range(R)), trace=trace)


def kernel(**inputs):
    res = run(inputs, stages=4, dbg=False, trace=False)
    parts = [res.results[c]['out'] for c in range(R)]
    full = np.concatenate(parts, axis=1)
    return full.reshape(B, S, V).astype(np.float32)


# revision 5
# speedup vs baseline: 2.3346x; 2.3346x over previous
"""Trainium2 Bass kernel for nn_MoRAPEForCausalLM (MoR expert-choice routing).

Self-contained. kernel(**inputs) -> np.ndarray [2, 2048, 32000] fp32.

Sharding (8 cores, SPMD single NEFF): tokens sharded (batch = core//4,
quarter = core%4); activations feature-major [D, T] in SBUF; K/V + routing
state exchanged via AllGather; device-side top-k (threshold bisection +
prefix-sum compaction + indirect DMA); lm_head vocab-sharded. Per-core
behavior via partition_id registers (dynamic DMA slices) + per-core small
inputs (attention-rank exp bias).

v2: all matmuls single-pass fp16 (weights + activation splits), full weight
packs passed per-core as inputs (no weight AllGather), resident [P,KD,1024]
weight tiles with 256KB+ DMAs, fused K+V collective per block, fp16 lm_head.
Routing logits/bisect/top-k stay exact fp32 on DVE.
"""
import math

import numpy as np

import concourse.bass as bass
import concourse.mybir as mybir
import concourse.tile as tile
from concourse import bacc
from concourse.bass import ts, ds
from concourse.bass_utils import run_bass_kernel_spmd
from concourse.expressions import smax
from concourse.masks import make_identity

P = 128
f32 = mybir.dt.float32
f32r = mybir.dt.float32r
f16 = mybir.dt.float16
i32 = mybir.dt.int32
AF = mybir.ActivationFunctionType
OP = mybir.AluOpType

B, S, D, H, DH, F, V = 2, 2048, 1024, 16, 64, 4096, 32000
R, NRANK = 8, 4
ALPHA, EPS = 0.1, 1e-6
KD, KF = D // P, F // P
T0 = B * S // R          # 512
T1 = T0 // 2             # 256
T2 = T0 // 4             # 128
VS = V // R              # 4000
ISQ = 1.0 / math.sqrt(DH)
E1 = DH + 1              # 65

BISECT_ITERS = 26
KGRP = 8
REPL = [list(range(R))]
NEG = -30.0

WSHAPES = {'wq': (D, D), 'wk': (D, D), 'wv': (D, D), 'wo': (D, D),
           'wg': (D, F), 'wu': (D, F), 'wd': (F, D)}
WNAMES = ('wq', 'wk', 'wv', 'wo', 'wg', 'wu', 'wd')
REFNAMES = {'wq': 'Wq', 'wk': 'Wk', 'wv': 'Wv', 'wo': 'Wo',
            'wg': 'Wg', 'wu': 'Wu', 'wd': 'Wd'}


def make_pack_meta():
    meta = {}
    for blk in range(6):
        items = []
        off = 0
        for wn in WNAMES:
            rows, cols = WSHAPES[wn]
            items.append((wn, rows, cols, off))
            off += (rows // R) * cols
        meta[blk] = (items, off)
    return meta


PACK_META = make_pack_meta()


class CX:
    pass


def load_wgroup(cx, pool, blk, wn, kbase, c0, cn, tag, nk=KD, bufs=2):
    """Resident [P, nk, cn] f16 weight tile; row-tiles kbase..kbase+nk,
    col slice [c0, c0+cn). One big DMA per row-tile."""
    nc = cx.nc
    items, _ = PACK_META[blk]
    for k, rows, cols, off in items:
        if k == wn:
            wt = pool.tile([P, nk, cn], f16, tag=tag, bufs=bufs,
                           name=f"{tag}_{cx.uid()}")
            rpr = rows // R
            for kk in range(nk):
                row0 = (kbase + kk) * P
                rank, rrow = row0 // rpr, row0 % rpr
                apv = cx.wpacks[blk][rank, ds(off + rrow * cols, P * cols)]
                apm = apv.rearrange("(p c) -> p c", c=cols)
                nc.sync.dma_start(wt[:, kk], apm[:, ds(c0, cn)])
            return wt
    raise KeyError(wn)


def split16(cx, pool, src_ap, tag, Tc, rows=P, bufs=1):
    nc = cx.nc
    hi = pool.tile([rows, Tc], f16, tag=f"{tag}h", bufs=bufs,
                   name=f"{tag}h_{cx.uid()}")
    nc.vector.tensor_copy(hi[:], src_ap)
    return hi


def rmsnorm(cx, pool, x_tiles, g_row, T, tag):
    nc = cx.nc
    sq = pool.tile([P, T], f32, tag="nsq", bufs=2, name=f"nsq_{cx.uid()}")
    ssum = cx.psC.tile([1, T], f32, tag="mis1", name=f"nss_{cx.uid()}")
    for ko in range(KD):
        nc.vector.tensor_mul(sq[:], x_tiles[ko][:], x_tiles[ko][:])
        nc.tensor.matmul(ssum[:], cx.ones_col[:], sq[:],
                         start=(ko == 0), stop=(ko == KD - 1))
    rms = pool.tile([1, T], f32, tag="nrm", bufs=1, name=f"nrm_{cx.uid()}")
    nc.vector.tensor_scalar(rms[:], ssum[:], 1.0 / D, EPS, op0=OP.mult, op1=OP.add)
    nc.scalar.activation(rms[:], rms[:], AF.Sqrt)
    rinv = pool.tile([1, T], f32, tag="nri", bufs=1, name=f"nri_{cx.uid()}")
    nc.vector.reciprocal(rinv[:], rms[:])
    bc = cx.psC.tile([P, T], f32, tag="mis2", name=f"nbc_{cx.uid()}")
    nc.tensor.matmul(bc[:], cx.ones_row[:], rinv[:], start=True, stop=True)
    bcs = pool.tile([P, T], f32, tag="nbcs", bufs=1, name=f"nbcs_{cx.uid()}")
    nc.vector.tensor_copy(bcs[:], bc[:])
    out = []
    for ko in range(KD):
        xn = pool.tile([P, T], f32, tag="nxn", bufs=2, name=f"nxn_{cx.uid()}")
        nc.vector.tensor_mul(xn[:], x_tiles[ko][:], bcs[:])
        nc.vector.tensor_tensor(
            xn[:, None, :], xn[:, None, :],
            cx.ln_sb[:, g_row, ko, None, None].to_broadcast([P, 1, T]), OP.mult)
        out.append(split16(cx, pool, xn[:], f"{tag}{ko}", T))
    return out


def linear_res(cx, wt, xin, T, Mtiles, out_cb):
    """out[m] = sum_ko wt[:,ko,m-slice].T @ xin[ko], Mtiles output tiles."""
    nc = cx.nc
    for mg in range(0, Mtiles, 2):
        pts = [cx.psA.tile([P, T], f32, tag=("ps" if mi == 0 else "sc"),
                           name=f"lps{mi}_{cx.uid()}") for mi in range(2)]
        for ko in range(KD):
            for mi in range(2):
                nc.tensor.matmul(pts[mi][:], wt[:, ko, ds((mg + mi) * P, P)],
                                 xin[ko][:], start=(ko == 0), stop=(ko == KD - 1))
        for mi in range(2):
            out_cb(mg + mi, pts[mi])


def llama_block(cx, dram, x_tiles, blk, T):
    nc, tc = cx.nc, cx.tc
    SK = T // P
    tg = f"b{blk}"
    KVL = D * T + T * H * E1

    with tc.tile_pool(name=f"bp{blk}", bufs=1) as bp:
        q_sp = [None] * KD
        kvloc = dram.tile([KVL], f16, tag=f"{tg}kv", name=f"{tg}kv")
        kloc = kvloc[ds(0, D * T)].rearrange("(d t) -> d t", t=T)
        vloc = kvloc[ds(D * T, T * H * E1)].rearrange("(t e) -> t e", e=H * E1)

        with tc.tile_pool(name=f"qk{blk}", bufs=2) as sp:
            xn = rmsnorm(cx, sp, x_tiles, 2 * blk, T, "xn")

            wqt = load_wgroup(cx, sp, blk, 'wq', 0, 0, D, "wqkv")

            def q_cb(mo, pt):
                q_sp[mo] = split16(cx, bp, pt[:], f"qs{mo}", T)

            linear_res(cx, wqt, xn, T, KD, q_cb)

            wkt = load_wgroup(cx, sp, blk, 'wk', 0, 0, D, "wqkv")

            def k_cb(mo, pt):
                kh = split16(cx, sp, pt[:], "kk", T, bufs=2)
                nc.sync.dma_start(kloc[ds(mo * P, P)], kh[:])

            linear_res(cx, wkt, xn, T, KD, k_cb)

            wvt = load_wgroup(cx, sp, blk, 'wv', 0, 0, D, "wqkv")
            for tt in range(SK):
                vsb = sp.tile([P, H * E1], f16, tag="vsb", bufs=2,
                              name=f"vsb_{cx.uid()}")
                nc.vector.memset(vsb[:], 1.0)
                for nc2 in range(D // 512):
                    pt = cx.psA.tile([P, 512], f32, tag="ps", name=f"vps_{cx.uid()}")
                    for ko in range(KD):
                        nc.tensor.matmul(pt[:], xn[ko][:, ts(tt, P)],
                                         wvt[:, ko, ds(nc2 * 512, 512)],
                                         start=(ko == 0), stop=(ko == KD - 1))
                    nh = 512 // DH
                    nc.vector.tensor_copy(
                        vsb[:, ds(nc2 * nh * E1, nh * E1)].rearrange(
                            "p (h e) -> p h e", e=E1)[:, :, :DH],
                        pt[:].rearrange("p (h e) -> p h e", e=DH))
                nc.sync.dma_start(vloc[ds(tt * P, P)], vsb[:])

        kvall = dram.tile([R, KVL], f16, tag=f"{tg}kva", name=f"{tg}kva",
                          addr_space="Shared")
        nc.gpsimd.collective_compute("AllGather", OP.bypass, replica_groups=REPL,
                                     ins=[kvloc[:].opt()], outs=[kvall[:].opt()])
        kvall_f = kvall[:].rearrange("r l -> (r l)")

        def kslice(src, d0):
            # [P, T] rows d0..d0+P of rank src's K
            return kvall_f[ds(src * KVL + d0 * T, P * T)].rearrange(
                "(p t) -> p t", t=T)

        def vslice(src, t0):
            # [P, H*E1] token rows t0..t0+P of rank src's V
            return kvall_f[ds(src * KVL + D * T + t0 * H * E1, P * H * E1)].rearrange(
                "(p e) -> p e", e=H * E1)

        attn_sp = [None] * KD
        with tc.tile_pool(name=f"at{blk}", bufs=2) as sp:
            vbufs = []
            for jrel in range(NRANK):
                src = cx.srcs[jrel]
                vb = sp.tile([P, SK, H * E1], f16, tag=f"vb{jrel}", bufs=1,
                             name=f"vb{jrel}_{cx.uid()}")
                for kk in range(SK):
                    nc.sync.dma_start(vb[:, kk], vslice(src, kk * P))
                vbufs.append(vb)
            for hp in range(H // 2):
                kbufs = []
                for jrel in range(NRANK):
                    src = cx.srcs[jrel]
                    kb = sp.tile([P, T], f16, tag=f"kb{jrel}", bufs=2,
                                 name=f"kb{jrel}_{cx.uid()}")
                    nc.sync.dma_start(kb[:], kslice(src, hp * P))
                    kbufs.append(kb)
                recip = sp.tile([33, T], f32, tag="rc", bufs=2, name=f"rc_{cx.uid()}")
                ovs = []
                for hpar in range(2):
                    h = 2 * hp + hpar
                    qrow = DH * hpar
                    rh = q_sp[hp][ds(qrow, DH)]
                    ov = cx.psB.tile([E1, T], f32, tag="ov",
                                     name=f"ov_{cx.uid()}")
                    total_sk = NRANK * SK
                    isk = 0
                    for jrel in range(NRANK):
                        for kk in range(SK):
                            sc = cx.psA.tile([P, T], f32, tag="sc",
                                             name=f"sc_{cx.uid()}")
                            nc.tensor.matmul(sc[:],
                                             kbufs[jrel][ds(qrow, DH), ts(kk, P)],
                                             rh, start=True, stop=True)
                            ex = sp.tile([P, T], f16, tag="ex", bufs=2,
                                         name=f"ex_{cx.uid()}")
                            if jrel == 0:
                                tmp = sp.tile([P, T], f32, tag="ext", bufs=2,
                                              name=f"ext_{cx.uid()}")
                                nc.scalar.activation(tmp[:], sc[:], AF.Exp, scale=ISQ)
                                nc.gpsimd.affine_select(
                                    ex[:], tmp[:], pattern=[[1, T]],
                                    compare_op=OP.is_ge, fill=0.0,
                                    base=-kk * P, channel_multiplier=-1)
                            else:
                                nc.scalar.activation(ex[:], sc[:], AF.Exp, scale=ISQ,
                                                     bias=cx.ab_sb[:, jrel:jrel + 1])
                            nc.tensor.matmul(ov[:],
                                             vbufs[jrel][:, kk, ds(h * E1, E1)],
                                             ex[:], start=(isk == 0),
                                             stop=(isk == total_sk - 1))
                            isk += 1
                    nc.vector.reciprocal(recip[ds(32 * hpar, 1)], ov[ds(DH, 1)])
                    ovs.append(ov)
                nbc = cx.psC.tile([P, T], f32, tag="mis2", name=f"nb_{cx.uid()}")
                nc.tensor.matmul(nbc[:], cx.sel2[:], recip[:], start=True, stop=True)
                nbs = sp.tile([P, T], f32, tag="nbs", bufs=2, name=f"nbs_{cx.uid()}")
                nc.vector.tensor_copy(nbs[:], nbc[:])
                at_f = sp.tile([P, T], f32, tag="atf", bufs=2, name=f"atf_{cx.uid()}")
                nc.vector.tensor_mul(at_f[ds(0, DH)], ovs[0][ds(0, DH)],
                                     nbs[ds(0, DH)])
                nc.vector.tensor_mul(at_f[ds(DH, DH)], ovs[1][ds(0, DH)],
                                     nbs[ds(DH, DH)])
                attn_sp[hp] = split16(cx, bp, at_f[:], f"as{hp}", T)

        with tc.tile_pool(name=f"op{blk}", bufs=2) as sp:
            wot = load_wgroup(cx, sp, blk, 'wo', 0, 0, D, "wot")

            def o_cb(mo, pt):
                nc.vector.tensor_add(x_tiles[mo][:], x_tiles[mo][:], pt[:])

            linear_res(cx, wot, attn_sp, T, KD, o_cb)

    with tc.tile_pool(name=f"ml{blk}", bufs=2) as sp:
        xn2 = rmsnorm(cx, sp, x_tiles, 2 * blk + 1, T, "xm")
        for g0 in range(0, KF, KGRP):
            wgt = load_wgroup(cx, sp, blk, 'wg', 0, g0 * P, KGRP * P, "wgt")
            wut = load_wgroup(cx, sp, blk, 'wu', 0, g0 * P, KGRP * P, "wut")
            gu_sp = [None] * KGRP
            for f0 in range(0, KGRP, 2):
                gps = [cx.psA.tile([P, T], f32, tag=t_, name=f"g{mi}_{cx.uid()}")
                       for mi, t_ in enumerate(("ps", "sc"))]
                ups = [cx.psB.tile([P, T], f32, tag="ov", name=f"u0_{cx.uid()}"),
                       cx.psC.tile([P, T], f32, tag="mis2", name=f"u1_{cx.uid()}")]
                for ko in range(KD):
                    xh = xn2[ko]
                    for mi in range(2):
                        nc.tensor.matmul(gps[mi][:],
                                         wgt[:, ko, ds((f0 + mi) * P, P)], xh[:],
                                         start=(ko == 0), stop=(ko == KD - 1))
                        nc.tensor.matmul(ups[mi][:],
                                         wut[:, ko, ds((f0 + mi) * P, P)], xh[:],
                                         start=(ko == 0), stop=(ko == KD - 1))
                for mi in range(2):
                    gs = sp.tile([P, T], f32, tag="gss", bufs=2,
                                 name=f"gss_{cx.uid()}")
                    nc.scalar.activation(gs[:], gps[mi][:], AF.Silu)
                    gu_f = sp.tile([P, T], f32, tag="guf", bufs=2,
                                   name=f"guf_{cx.uid()}")
                    nc.vector.tensor_mul(gu_f[:], gs[:], ups[mi][:])
                    gu_sp[f0 + mi] = split16(cx, sp, gu_f[:], f"gu{f0 + mi}", T)
            wdt = load_wgroup(cx, sp, blk, 'wd', g0, 0, D, "wdt", nk=KGRP)
            for mg in range(0, KD, 2):
                pts = [cx.psA.tile([P, T], f32, tag=t_, name=f"d{mi}_{cx.uid()}")
                       for mi, t_ in enumerate(("ps", "sc"))]
                for k2 in range(KGRP):
                    for mi in range(2):
                        nc.tensor.matmul(pts[mi][:],
                                         wdt[:, k2, ds((mg + mi) * P, P)],
                                         gu_sp[k2][:],
                                         start=(k2 == 0), stop=(k2 == KGRP - 1))
                for mi in range(2):
                    nc.vector.tensor_add(x_tiles[mg + mi][:],
                                         x_tiles[mg + mi][:], pts[mi][:])


def dve_matvec(cx, pool, x_tiles, rw_row, T):
    nc = cx.nc
    acc = pool.tile([P, T], f32, tag="mvac", bufs=1, name=f"mvac_{cx.uid()}")
    tmp = pool.tile([P, T], f32, tag="mvtp", bufs=1, name=f"mvtp_{cx.uid()}")
    for ko in range(KD):
        dst = acc if ko == 0 else tmp
        nc.vector.tensor_tensor(
            dst[:, None, :], x_tiles[ko][:, None, :],
            cx.rw_sb[:, rw_row, ko, None, None].to_broadcast([P, 1, T]), OP.mult)
        if ko > 0:
            nc.vector.tensor_add(acc[:], acc[:], tmp[:])
    pt = cx.psC.tile([1, T], f32, tag="mis1", name=f"mv_{cx.uid()}")
    nc.tensor.matmul(pt[:], cx.ones_col[:], acc[:], start=True, stop=True)
    lg = pool.tile([1, T], f32, tag="mvlg", bufs=1, name=f"mvlg_{cx.uid()}")
    nc.vector.tensor_copy(lg[:], pt[:])
    return lg


def bisect_mask(cx, pool, lall_flat, Sb, ktarget):
    nc = cx.nc
    nb = Sb // P
    lg = pool.tile([P, B, nb], f32, tag="bilg", bufs=1, name=f"bilg_{cx.uid()}")
    for bb in range(B):
        nc.sync.dma_start(lg[:, bb],
                          lall_flat[ds(bb * Sb, Sb)].rearrange("(p c) -> p c", c=nb))
    lo = pool.tile([P, B, nb], f32, tag="bilo", bufs=1, name=f"bilo_{cx.uid()}")
    hi = pool.tile([P, B, nb], f32, tag="bihi", bufs=1, name=f"bihi_{cx.uid()}")
    nc.vector.memset(lo[:], -16.0)
    nc.vector.memset(hi[:], 16.0)
    mid = pool.tile([P, B, nb], f32, tag="bimd", bufs=1, name=f"bimd_{cx.uid()}")
    cmp = pool.tile([P, B, nb], f32, tag="bicp", bufs=1, name=f"bicp_{cx.uid()}")
    red = pool.tile([P, B, 1], f32, tag="bird", bufs=1, name=f"bird_{cx.uid()}")
    cnt_sb = pool.tile([1, B], f32, tag="bict", bufs=1, name=f"bict_{cx.uid()}")
    pred = pool.tile([P, B], f32, tag="bipd", bufs=1, name=f"bipd_{cx.uid()}")
    dlt = pool.tile([P, B, nb], f32, tag="bidl", bufs=1, name=f"bidl_{cx.uid()}")
    for _ in range(BISECT_ITERS):
        nc.vector.tensor_add(mid[:], lo[:], hi[:])
        nc.vector.tensor_scalar_mul(mid[:], mid[:], 0.5)
        nc.vector.tensor_tensor(cmp[:], lg[:], mid[:], OP.is_gt)
        nc.vector.tensor_reduce(red[:], cmp[:], axis=mybir.AxisListType.X, op=OP.add)
        cnt = cx.psC.tile([1, B], f32, tag="mis1", name=f"bic_{cx.uid()}")
        nc.tensor.matmul(cnt[:], cx.ones_col[:], red[:, :, 0], start=True, stop=True)
        nc.vector.tensor_copy(cnt_sb[:], cnt[:])
        cbc = cx.psC.tile([P, B], f32, tag="mis2", name=f"bib_{cx.uid()}")
        nc.tensor.matmul(cbc[:], cx.ones_row[:], cnt_sb[:], start=True, stop=True)
        nc.vector.tensor_scalar(pred[:], cbc[:], float(ktarget), None, op0=OP.is_ge)
        # lo += pred * (mid - lo); hi += (1 - pred) * (mid - hi)
        nc.vector.tensor_sub(dlt[:], mid[:], lo[:])
        nc.vector.tensor_tensor(dlt[:], dlt[:],
                                pred[:, :, None].to_broadcast([P, B, nb]), OP.mult)
        nc.vector.tensor_add(lo[:], lo[:], dlt[:])
        nc.vector.tensor_scalar(pred[:], cbc[:], float(ktarget), None, op0=OP.is_lt)
        nc.vector.tensor_sub(dlt[:], mid[:], hi[:])
        nc.vector.tensor_tensor(dlt[:], dlt[:],
                                pred[:, :, None].to_broadcast([P, B, nb]), OP.mult)
        nc.vector.tensor_add(hi[:], hi[:], dlt[:])
    mask = pool.tile([P, B, nb], f32, tag="bimk", bufs=1, name=f"bimk_{cx.uid()}")
    nc.vector.tensor_tensor(mask[:], lg[:], lo[:], OP.is_gt)
    return mask


def cumsum_pos(cx, pool, dram, mask, Sb, ksel, tag):
    nc = cx.nc
    nb = Sb // P
    a = mask
    s, pp = 1, 0
    while s < nb:
        bt = pool.tile([P, B, nb], f32, tag=f"cs{pp % 2}", bufs=1,
                       name=f"cs_{cx.uid()}")
        nc.vector.tensor_copy(bt[:, :, :s], a[:, :, :s])
        nc.vector.tensor_add(bt[:, :, s:], a[:, :, s:], a[:, :, :nb - s])
        a = bt
        s *= 2
        pp += 1
    tot = pool.tile([P, B], f32, tag="cstt", bufs=1, name=f"cstt_{cx.uid()}")
    nc.vector.tensor_copy(tot[:], a[:, :, nb - 1])
    ppf = cx.psC.tile([P, B], f32, tag="mis2", name=f"csp_{cx.uid()}")
    nc.tensor.matmul(ppf[:], cx.triu[:], tot[:], start=True, stop=True)
    cs = pool.tile([P, B, nb], f32, tag="cscs", bufs=1, name=f"cscs_{cx.uid()}")
    nc.vector.tensor_tensor(cs[:], a[:], ppf[:, :, None].to_broadcast([P, B, nb]),
                            OP.add)
    csd = dram.tile([B, Sb], f32, tag=f"{tag}csd", name=f"{tag}csd")
    nc.sync.dma_start(csd[:].rearrange("b (p c) -> p b c", p=P), cs[:])
    posd = dram.tile([B * ksel, 1], f32, tag=f"{tag}posd", name=f"{tag}posd")
    for bb in range(B):
        csrow = pool.tile([1, Sb], f32, tag="cscr", bufs=1, name=f"cscr_{cx.uid()}")
        nc.sync.dma_start(csrow[:], csd[bb, None, :])
        cbc = pool.tile([P, Sb], f32, tag="cscb", bufs=1, name=f"cscb_{cx.uid()}")
        for ch in range(0, Sb, 512):
            w = min(512, Sb - ch)
            pt = cx.psC.tile([P, 512], f32, tag="mis2", name=f"csq_{cx.uid()}")
            nc.tensor.matmul(pt[:, :w], cx.ones_row[:], csrow[:, ds(ch, w)],
                             start=True, stop=True)
            nc.vector.tensor_copy(cbc[:, ds(ch, w)], pt[:, :w])
        for rt in range(ksel // P):
            rcol = pool.tile([P, 1], f32, tag="csrc", bufs=2, name=f"csrc_{cx.uid()}")
            nc.vector.tensor_scalar_add(rcol[:], cx.iota_f[:], float(rt * P))
            cmp = pool.tile([P, Sb], f32, tag="cscm", bufs=2, name=f"cscm_{cx.uid()}")
            nc.vector.tensor_tensor(cmp[:], cbc[:], rcol[:].to_broadcast([P, Sb]),
                                    OP.is_le)
            red = pool.tile([P, 1], f32, tag="csrd", bufs=2, name=f"csrd_{cx.uid()}")
            nc.vector.tensor_reduce(red[:], cmp[:], axis=mybir.AxisListType.X,
                                    op=OP.add)
            nc.sync.dma_start(posd[ds(bb * ksel + rt * P, P)], red[:])
    return posd


def to_tok_dram(cx, pool, dtile, x_tiles, T):
    nc = cx.nc
    for tt in range(T // P):
        asm = pool.tile([P, D], f32, tag="tkas", bufs=2, name=f"tkas_{cx.uid()}")
        for ko in range(KD):
            tr = cx.psC.tile([P, P], f32, tag="mis2", name=f"tktr_{cx.uid()}")
            nc.tensor.transpose(tr[:], x_tiles[ko][:, ts(tt, P)], cx.ident[:])
            nc.vector.tensor_copy(asm[:, ts(ko, P)], tr[:])
        nc.sync.dma_start(dtile[ds(tt * P, P)], asm[:])


def gather_sel(cx, pool, res, src_flat, posd, T, boff_col, rtag):
    nc = cx.nc
    myoff = cx.pid * T
    xt = [res.tile([P, T], f32, tag=f"{rtag}{ko}", name=f"{rtag}{ko}")
          for ko in range(KD)]
    for u in range(T // P):
        pv = pool.tile([P, 1], f32, tag="gspv", bufs=2, name=f"gspv_{cx.uid()}")
        nc.sync.dma_start(pv[:], posd[ds(myoff + u * P, P)])
        nc.vector.tensor_scalar(pv[:], pv[:], boff_col, None, op0=OP.add)
        pi = pool.tile([P, 1], i32, tag="gspi", bufs=2, name=f"gspi_{cx.uid()}")
        nc.vector.tensor_copy(pi[:], pv[:])
        g = pool.tile([P, D], f32, tag="gsg", bufs=2, name=f"gsg_{cx.uid()}")
        nc.gpsimd.indirect_dma_start(
            out=g[:], out_offset=None, in_=src_flat,
            in_offset=bass.IndirectOffsetOnAxis(ap=pi[:, :1], axis=0))
        for ko in range(KD):
            tr = cx.psC.tile([P, P], f32, tag="mis2", name=f"gstr_{cx.uid()}")
            nc.tensor.transpose(tr[:], g[:, ts(ko, P)], cx.ident[:])
            nc.vector.tensor_copy(xt[ko][:, ts(u, P)], tr[:])
    return xt


def topw_bcast(cx, pool, sel_in, rw_row, T):
    nc = cx.nc
    lgs = dve_matvec(cx, pool, sel_in, rw_row, T)
    tw = pool.tile([1, T], f32, tag="twr", bufs=1, name=f"twr_{cx.uid()}")
    nc.scalar.activation(tw[:], lgs[:], AF.Sigmoid)
    nc.vector.tensor_scalar_mul(tw[:], tw[:], ALPHA)
    pt = cx.psC.tile([P, T], f32, tag="mis2", name=f"twp_{cx.uid()}")
    nc.tensor.matmul(pt[:], cx.ones_row[:], tw[:], start=True, stop=True)
    twb = pool.tile([P, T], f32, tag="twb", bufs=1, name=f"twb_{cx.uid()}")
    nc.vector.tensor_copy(twb[:], pt[:])
    return twb


def build_program(stages=4, dbg=False):
    nc = bacc.Bacc("TRN2", target_bir_lowering=False)
    cx = CX()
    cx.nc = nc
    cx._u = 0

    def uid():
        cx._u += 1
        return cx._u
    cx.uid = uid

    innames = ["h0T", "ln", "rw", "abias", "fvec", "sel2c"]
    h0T = nc.declare_dram_parameter("h0T", [D, T0], f32, isOutput=False)
    lnp = nc.declare_dram_parameter("ln", [13, D], f32, isOutput=False)
    rwp = nc.declare_dram_parameter("rw", [2, D], f32, isOutput=False)
    abp = nc.declare_dram_parameter("abias", [NRANK, P], f32, isOutput=False)
    fvp = nc.declare_dram_parameter("fvec", [P, 4], f32, isOutput=False)
    s2p = nc.declare_dram_parameter("sel2c", [33, P], f32, isOutput=False)
    nblk = 6 if stages >= 3 else (3 if stages >= 2 else 1)
    wparams = {}
    for blk in range(nblk):
        items, shard = PACK_META[blk]
        wparams[blk] = nc.declare_dram_parameter(f"wpack{blk}", [R, shard], f16,
                                                 isOutput=False)
        innames.append(f"wpack{blk}")
    out = embT = None
    if stages >= 4:
        embT = nc.declare_dram_parameter("embT", [D, VS], f16, isOutput=False)
        out = nc.declare_dram_parameter("out", [B * S, VS], f32, isOutput=True)
        innames.append("embT")
    dbg_o = {}

    def dbg_out(nm, shp):
        dbg_o[nm] = nc.declare_dram_parameter(nm, shp, f32, isOutput=True)
        return dbg_o[nm]

    with tile.TileContext(nc) as tc:
        cx.tc = tc
        with (
            tc.tile_pool(name="const", bufs=1) as cst,
            tc.tile_pool(name="res", bufs=1) as res,
            tc.tile_pool(name="psA", bufs=2, space="PSUM") as psA,
            tc.tile_pool(name="psB", bufs=2, space="PSUM") as psB,
            tc.tile_pool(name="psC", bufs=1, space="PSUM") as psC,
            tc.tile_pool(name="dram", bufs=1, space="DRAM") as dram,
        ):
            cx.psA, cx.psB, cx.psC = psA, psB, psC

            cx.ones_col = cst.tile([P, 1], f32, name="ones_col")
            nc.vector.memset(cx.ones_col[:], 1.0)
            cx.ones_row = cst.tile([1, P], f32, name="ones_row")
            nc.vector.memset(cx.ones_row[:], 1.0)
            cx.sel2 = cst.tile([33, P], f32, name="sel2")
            nc.sync.dma_start(cx.sel2[:], s2p.ap())
            cx.ident = cst.tile([P, P], f32, name="ident")
            make_identity(nc, cx.ident[:])
            onespp = cst.tile([P, P], f32, name="onespp")
            nc.vector.memset(onespp[:], 1.0)
            cx.triu = cst.tile([P, P], f32, name="triu")
            nc.gpsimd.affine_select(cx.triu[:], onespp[:], pattern=[[1, P]],
                                    compare_op=OP.is_ge, fill=0.0, base=-1,
                                    channel_multiplier=-1)
            iota_i = cst.tile([P, 1], i32, name="iota_i")
            nc.gpsimd.iota(iota_i[:], pattern=[[0, 1]], base=0, channel_multiplier=1)
            cx.iota_f = cst.tile([P, 1], f32, name="iota_f")
            nc.vector.tensor_copy(cx.iota_f[:], iota_i[:])
            cx.ln_sb = cst.tile([P, 13, KD], f32, name="ln_sb")
            nc.sync.dma_start(cx.ln_sb[:],
                              lnp.ap().rearrange("r (ko p) -> p r ko", p=P))
            cx.rw_sb = cst.tile([P, 2, KD], f32, name="rw_sb")
            nc.sync.dma_start(cx.rw_sb[:],
                              rwp.ap().rearrange("r (ko p) -> p r ko", p=P))
            cx.ab_sb = cst.tile([P, NRANK], f32, name="ab_sb")
            nc.sync.dma_start(cx.ab_sb[:], abp.ap().rearrange("j p -> p j"))
            cx.fv_sb = cst.tile([P, 4], f32, name="fv_sb")
            nc.sync.dma_start(cx.fv_sb[:], fvp.ap())

            pid = nc.sync.partition_id()
            cx.pid = pid
            qreg = pid % NRANK
            base = pid - qreg
            cx.srcs = [smax(pid - j, base) for j in range(NRANK)]

            cx.wpacks = {}
            for blk in range(nblk):
                cx.wpacks[blk] = wparams[blk].ap()

            # ---- stage 1: block 0 + recursion-0 routing
            with tc.tile_pool(name="st1", bufs=1) as st1:
                x = [st1.tile([P, T0], f32, tag=f"xa{ko}", name=f"xa{ko}")
                     for ko in range(KD)]
                h0ap = h0T.ap().rearrange("(ko p) t -> p ko t", p=P)
                for ko in range(KD):
                    nc.sync.dma_start(x[ko][:], h0ap[:, ko])
                llama_block(cx, dram, x, 0, T0)

                with tc.tile_pool(name="rt0", bufs=2) as rp:
                    lg0 = dve_matvec(cx, rp, x, 0, T0)
                    lloc = dram.tile([1, T0], f32, tag="lloc0", name="lloc0")
                    nc.sync.dma_start(lloc[:], lg0[:])
                    lall = dram.tile([R, 1, T0], f32, tag="lall0", name="lall0",
                                     addr_space="Shared")
                    nc.gpsimd.collective_compute(
                        "AllGather", OP.bypass, replica_groups=REPL,
                        ins=[lloc[:].opt()], outs=[lall[:].opt()])
                    htl = dram.tile([T0, D], f32, tag="htl", name="htl")
                    to_tok_dram(cx, rp, htl, x, T0)
                    hta = dram.tile([R, T0, D], f32, tag="hta", name="hta",
                                    addr_space="Shared")
                    nc.gpsimd.collective_compute(
                        "AllGather", OP.bypass, replica_groups=REPL,
                        ins=[htl[:].opt()], outs=[hta[:].opt()])
                    cx.hta_r = hta[:].rearrange("r t d -> (r t) d")

                    mask0 = bisect_mask(cx, rp,
                                        lall[:].rearrange("r o t -> (r o t)"),
                                        S, S // 2)
                    posd0 = cumsum_pos(cx, rp, dram, mask0, S, S // 2, "c0")
                    seli = gather_sel(cx, rp, res, cx.hta_r, posd0, T1,
                                      cx.fv_sb[:, 0:1], "sli")
                    if dbg:
                        d1 = dbg_out("dbg_h0b", [T0, D])
                        nc.sync.dma_start(d1.ap(), htl[:])
                        d2 = dbg_out("dbg_lg", [1, T0])
                        nc.sync.dma_start(d2.ap(), lloc[:])
                        d3 = dbg_out("dbg_pos", [B * S // 2, 1])
                        nc.sync.dma_start(d3.ap(), posd0[:])
                        d4 = dbg_out("dbg_selT", [D, T1])
                        d4r = d4.ap().rearrange("(ko p) t -> p ko t", p=P)
                        for ko in range(KD):
                            nc.sync.dma_start(d4r[:, ko], seli[ko][:])

            if stages >= 2:
                with tc.tile_pool(name="st2", bufs=1) as st2:
                    sel = [st2.tile([P, T1], f32, tag=f"sl{ko}", name=f"sl{ko}")
                           for ko in range(KD)]
                    for ko in range(KD):
                        nc.vector.tensor_copy(sel[ko][:], seli[ko][:])
                    llama_block(cx, dram, sel, 1, T1)
                    llama_block(cx, dram, sel, 2, T1)
                    with tc.tile_pool(name="rt1", bufs=2) as rp:
                        twb0 = topw_bcast(cx, rp, seli, 0, T1)
                        x1 = [res.tile([P, T1], f32, tag=f"x1{ko}", name=f"x1{ko}")
                              for ko in range(KD)]
                        for ko in range(KD):
                            nc.vector.tensor_mul(x1[ko][:], sel[ko][:], twb0[:])
                            nc.vector.tensor_add(x1[ko][:], x1[ko][:], seli[ko][:])
                        lg1 = dve_matvec(cx, rp, x1, 1, T1)
                        lloc1 = dram.tile([1, T1], f32, tag="lloc1", name="lloc1")
                        nc.sync.dma_start(lloc1[:], lg1[:])
                        lall1 = dram.tile([R, 1, T1], f32, tag="lall1",
                                          name="lall1", addr_space="Shared")
                        nc.gpsimd.collective_compute(
                            "AllGather", OP.bypass, replica_groups=REPL,
                            ins=[lloc1[:].opt()], outs=[lall1[:].opt()])
                        x1l = dram.tile([T1, D], f32, tag="x1l", name="x1l")
                        to_tok_dram(cx, rp, x1l, x1, T1)
                        x1a = dram.tile([R, T1, D], f32, tag="x1a", name="x1a",
                                        addr_space="Shared")
                        nc.gpsimd.collective_compute(
                            "AllGather", OP.bypass, replica_groups=REPL,
                            ins=[x1l[:].opt()], outs=[x1a[:].opt()])
                        cx.x1a_r = x1a[:].rearrange("r t d -> (r t) d")

                        mask1 = bisect_mask(cx, rp,
                                            lall1[:].rearrange("r o t -> (r o t)"),
                                            S // 2, S // 4)
                        posd1 = cumsum_pos(cx, rp, dram, mask1, S // 2, S // 4, "c1")
                        sl1i = gather_sel(cx, rp, res, cx.x1a_r, posd1, T2,
                                          cx.fv_sb[:, 1:2], "s1i")
                        if dbg:
                            d5 = dbg_out("dbg_x1", [T1, D])
                            nc.sync.dma_start(d5.ap(), x1l[:])
                            d6 = dbg_out("dbg_pos1", [B * S // 4, 1])
                            nc.sync.dma_start(d6.ap(), posd1[:])

            if stages >= 3:
                with tc.tile_pool(name="st3", bufs=1) as st3:
                    sl1 = [st3.tile([P, T2], f32, tag=f"sm{ko}", name=f"sm{ko}")
                           for ko in range(KD)]
                    for ko in range(KD):
                        nc.vector.tensor_copy(sl1[ko][:], sl1i[ko][:])
                    llama_block(cx, dram, sl1, 3, T2)
                    llama_block(cx, dram, sl1, 4, T2)
                    with tc.tile_pool(name="rt2", bufs=2) as rp:
                        twb1 = topw_bcast(cx, rp, sl1i, 1, T2)
                        z = [st3.tile([P, T2], f32, tag=f"zz{ko}", name=f"zz{ko}")
                             for ko in range(KD)]
                        for ko in range(KD):
                            nc.vector.tensor_mul(z[ko][:], sl1[ko][:], twb1[:])
                            nc.vector.tensor_add(z[ko][:], z[ko][:], sl1i[ko][:])
                        zl = dram.tile([T2, D], f32, tag="zl", name="zl")
                        to_tok_dram(cx, rp, zl, z, T2)
                        za = dram.tile([R, T2, D], f32, tag="za", name="za",
                                       addr_space="Shared")
                        nc.gpsimd.collective_compute(
                            "AllGather", OP.bypass, replica_groups=REPL,
                            ins=[zl[:].opt()], outs=[za[:].opt()])
                        za_r = za[:].rearrange("r t d -> (r t) d")

                        h2loc = dram.tile([R * T0, D], f32, tag="h2loc",
                                          name="h2loc")
                        nc.sync.dma_start(h2loc[:], cx.hta_r)
                        cx.h2_r = h2loc[:]

                        for ch in range(B * S // 2 // P):
                            bb = ch // (S // 2 // P)
                            ssb = rp.tile([P, D], f32, tag="scx", bufs=2,
                                          name=f"scx_{cx.uid()}")
                            nc.sync.dma_start(ssb[:], cx.x1a_r[ds(ch * P, P)])
                            pv = rp.tile([P, 1], f32, tag="scp", bufs=2,
                                         name=f"scp_{cx.uid()}")
                            nc.sync.dma_start(pv[:], posd0[ds(ch * P, P)])
                            nc.vector.tensor_scalar_add(pv[:], pv[:], float(bb * S))
                            pi = rp.tile([P, 1], i32, tag="sci", bufs=2,
                                         name=f"sci_{cx.uid()}")
                            nc.vector.tensor_copy(pi[:], pv[:])
                            nc.gpsimd.indirect_dma_start(
                                out=cx.h2_r, out_offset=bass.IndirectOffsetOnAxis(
                                    ap=pi[:, :1], axis=0),
                                in_=ssb[:], in_offset=None)
                        for ch in range(B * S // 4 // P):
                            bb = ch // (S // 4 // P)
                            ssb = rp.tile([P, D], f32, tag="scz", bufs=2,
                                          name=f"scz_{cx.uid()}")
                            nc.sync.dma_start(ssb[:], za_r[ds(ch * P, P)])
                            p1 = rp.tile([P, 1], f32, tag="sc1", bufs=2,
                                         name=f"sc1_{cx.uid()}")
                            nc.sync.dma_start(p1[:], posd1[ds(ch * P, P)])
                            nc.vector.tensor_scalar_add(p1[:], p1[:],
                                                        float(bb * (S // 2)))
                            p1i = rp.tile([P, 1], i32, tag="sc2", bufs=2,
                                          name=f"sc2_{cx.uid()}")
                            nc.vector.tensor_copy(p1i[:], p1[:])
                            p0 = rp.tile([P, 1], f32, tag="sc3", bufs=2,
                                         name=f"sc3_{cx.uid()}")
                            nc.gpsimd.indirect_dma_start(
                                out=p0[:], out_offset=None, in_=posd0[:],
                                in_offset=bass.IndirectOffsetOnAxis(
                                    ap=p1i[:, :1], axis=0))
                            nc.vector.tensor_scalar_add(p0[:], p0[:], float(bb * S))
                            p0i = rp.tile([P, 1], i32, tag="sc4", bufs=2,
                                          name=f"sc4_{cx.uid()}")
                            nc.vector.tensor_copy(p0i[:], p0[:])
                            nc.gpsimd.indirect_dma_start(
                                out=cx.h2_r, out_offset=bass.IndirectOffsetOnAxis(
                                    ap=p0i[:, :1], axis=0),
                                in_=ssb[:], in_offset=None)
                        if dbg:
                            d7 = dbg_out("dbg_h2", [T0, D])
                            nc.sync.dma_start(d7.ap(), cx.h2_r[ds(cx.pid * T0, T0)])

            if stages >= 4:
                with tc.tile_pool(name="st4", bufs=1) as st4:
                    x5 = [st4.tile([P, T0], f32, tag=f"x5{ko}", name=f"x5{ko}")
                          for ko in range(KD)]
                    with tc.tile_pool(name="ld5", bufs=2) as rp:
                        for tt in range(T0 // P):
                            tkb = rp.tile([P, D], f32, tag="h2t", bufs=2,
                                          name=f"h2t_{cx.uid()}")
                            nc.sync.dma_start(tkb[:],
                                              cx.h2_r[ds(cx.pid * T0 + tt * P, P)])
                            for ko in range(KD):
                                tr = cx.psC.tile([P, P], f32, tag="mis2",
                                                 name=f"h2r_{cx.uid()}")
                                nc.tensor.transpose(tr[:], tkb[:, ts(ko, P)],
                                                    cx.ident[:])
                                nc.vector.tensor_copy(x5[ko][:, ts(tt, P)], tr[:])
                    llama_block(cx, dram, x5, 5, T0)
                    hfl = dram.tile([P, KD, T0], f16, tag="hfl", name="hfl")
                    with tc.tile_pool(name="fn5", bufs=2) as rp:
                        hfn = rmsnorm(cx, rp, x5, 12, T0, "hf")
                        for ko in range(KD):
                            nc.sync.dma_start(hfl[:, ko], hfn[ko][:])
                    hfa = dram.tile([R, P, KD, T0], f16, tag="hfa", name="hfa",
                                    addr_space="Shared")
                    nc.gpsimd.collective_compute(
                        "AllGather", OP.bypass, replica_groups=REPL,
                        ins=[hfl[:].opt()], outs=[hfa[:].opt()])
                with tc.tile_pool(name="hd", bufs=1) as hd:
                    NCH = 4
                    CH = VS // NCH  # 1000
                    for ch in range(NCH):
                        et = hd.tile([P, KD, CH], f16, tag="et", bufs=2,
                                     name=f"et_{cx.uid()}")
                        for ko in range(KD):
                            nc.sync.dma_start(
                                et[:, ko],
                                embT.ap()[ds(ko * P, P), ds(ch * CH, CH)])
                        for rr in range(R):
                            hl = hd.tile([P, KD, T0], f16, tag="hl", bufs=2,
                                         name=f"hl_{cx.uid()}")
                            nc.sync.dma_start(hl[:], hfa[rr])
                            for tt in range(T0 // P):
                                for hf2 in range(2):
                                    pt = cx.psA.tile([P, 500], f32, tag="ps",
                                                     name=f"hd_{cx.uid()}")
                                    for ko in range(KD):
                                        nc.tensor.matmul(
                                            pt[:], hl[:, ko, ts(tt, P)],
                                            et[:, ko, ds(hf2 * 500, 500)],
                                            start=(ko == 0), stop=(ko == KD - 1))
                                    ot = hd.tile([P, 500], f32, tag="hot", bufs=3,
                                                 name=f"hot_{cx.uid()}")
                                    nc.vector.tensor_copy(ot[:], pt[:])
                                    nc.sync.dma_start(
                                        out.ap()[ds(rr * T0 + tt * P, P),
                                                 ds(ch * CH + hf2 * 500, 500)],
                                        ot[:])
    nc.finalize()
    return nc, innames, list(dbg_o)


# ----------------------------------------------------------------------- host

_CACHE = {}


def _prepare_inmaps(inputs, stages):
    input_ids = np.asarray(inputs['input_ids'])
    embed = np.asarray(inputs['embed'], dtype=np.float32)
    pos_emb = np.asarray(inputs['pos_emb'], dtype=np.float32)
    h0 = embed[input_ids] + pos_emb[None, :, :]
    ln = np.empty((13, D), np.float32)
    for i in range(6):
        ln[2 * i] = inputs['ln1'][i]
        ln[2 * i + 1] = inputs['ln2'][i]
    ln[12] = inputs['final_norm']
    rw = np.asarray(inputs['router_w'], dtype=np.float32)

    nblk = 6 if stages >= 3 else (3 if stages >= 2 else 1)
    packs = {}
    for blk in range(nblk):
        items, shard = PACK_META[blk]
        pk = np.empty((R, shard), np.float16)
        for key, rows, cols, off in items:
            W = np.asarray(inputs[REFNAMES[key]][blk], dtype=np.float32)
            rpr = rows // R
            n = rpr * cols
            Wm = W.astype(np.float16).reshape(R, n)
            pk[:, off:off + n] = Wm
        packs[blk] = pk

    embT16 = None
    in_maps = []
    for c in range(R):
        b, q = c // NRANK, c % NRANK
        m = {}
        sl = h0[b, q * T0:(q + 1) * T0]
        m['h0T'] = np.ascontiguousarray(sl.T)
        m['ln'] = ln
        m['rw'] = rw
        ab = np.zeros((NRANK, P), np.float32)
        for j in range(NRANK):
            if j > q:
                ab[j] = NEG
        m['abias'] = ab
        m['fvec'] = np.tile(np.array([[b * S, b * (S // 2), 0, 0]], np.float32),
                            (P, 1))
        s2 = np.zeros((33, P), np.float32)
        s2[0, :DH] = 1.0
        s2[32, DH:] = 1.0
        m['sel2c'] = s2
        for blk in range(nblk):
            m[f'wpack{blk}'] = packs[blk]
        if stages >= 4:
            m['embT'] = np.ascontiguousarray(
                embed[c * VS:(c + 1) * VS].T.astype(np.float16))
        in_maps.append(m)
    return in_maps


def run(inputs, stages=4, dbg=False, trace=False):
    key = (stages, dbg)
    if key not in _CACHE:
        _CACHE[key] = build_program(stages, dbg)
    nc, innames, dbgnames = _CACHE[key]
    in_maps = _prepare_inmaps(inputs, stages)
    return run_bass_kernel_spmd(nc, in_maps, core_ids=list(range(R)), trace=trace)


def kernel(**inputs):
    res = run(inputs, stages=4, dbg=False, trace=False)
    parts = [res.results[c]['out'] for c in range(R)]
    full = np.concatenate(parts, axis=1)
    return full.reshape(B, S, V).astype(np.float32)


# revision 16
# speedup vs baseline: 2.4387x; 1.0446x over previous
"""Trainium2 Bass kernel for nn_MoRAPEForCausalLM (MoR expert-choice routing).

Self-contained. kernel(**inputs) -> np.ndarray [2, 2048, 32000] fp32.

Sharding (8 cores, SPMD single NEFF): tokens sharded (batch = core//4,
quarter = core%4); activations feature-major [D, T] in SBUF; K/V + routing
state exchanged via AllGather; device-side top-k (threshold bisection +
prefix-sum compaction + indirect DMA); lm_head vocab-sharded. Per-core
behavior via partition_id registers (dynamic DMA slices) + per-core small
inputs (attention-rank exp bias).

v2: all matmuls single-pass fp16 (weights + activation splits), full weight
packs passed per-core as inputs (no weight AllGather), resident [P,KD,1024]
weight tiles with 256KB+ DMAs, fused K+V collective per block, fp16 lm_head.
Routing logits/bisect/top-k stay exact fp32 on DVE.
"""
import math

import numpy as np

import concourse.bass as bass
import concourse.mybir as mybir
import concourse.tile as tile
from concourse import bacc
from concourse.bass import ts, ds
from concourse.bass_utils import run_bass_kernel_spmd
from concourse.expressions import smax
from concourse.masks import make_identity

P = 128
f32 = mybir.dt.float32
f32r = mybir.dt.float32r
f16 = mybir.dt.float16
i32 = mybir.dt.int32
AF = mybir.ActivationFunctionType
OP = mybir.AluOpType

B, S, D, H, DH, F, V = 2, 2048, 1024, 16, 64, 4096, 32000
R, NRANK = 8, 4
ALPHA, EPS = 0.1, 1e-6
KD, KF = D // P, F // P
T0 = B * S // R          # 512
T1 = T0 // 2             # 256
T2 = T0 // 4             # 128
VS = V // R              # 4000
ISQ = 1.0 / math.sqrt(DH)
E1 = DH + 1              # 65

KGRP = 8
REPL = [list(range(R))]
REPL4 = [[0, 1, 2, 3], [4, 5, 6, 7]]
NEG = -30.0

WSHAPES = {'wq': (D, D), 'wk': (D, D), 'wv': (D, D), 'wo': (D, D),
           'wg': (D, F), 'wu': (D, F), 'wd': (F, D)}
WNAMES = ('wq', 'wk', 'wv', 'wo', 'wg', 'wu', 'wd')
REFNAMES = {'wq': 'Wq', 'wk': 'Wk', 'wv': 'Wv', 'wo': 'Wo',
            'wg': 'Wg', 'wu': 'Wu', 'wd': 'Wd'}


def make_pack_meta():
    meta = {}
    for blk in range(6):
        items = []
        off = 0
        for wn in WNAMES:
            rows, cols = WSHAPES[wn]
            items.append((wn, rows, cols, off))
            off += (rows // R) * cols
        meta[blk] = (items, off)
    return meta


PACK_META = make_pack_meta()


class CX:
    pass


def load_wgroup(cx, pool, blk, wn, kbase, c0, cn, tag, nk=KD, bufs=2):
    """Resident [P, nk, cn] f16 weight tile; row-tiles kbase..kbase+nk,
    col slice [c0, c0+cn). One big DMA per row-tile."""
    nc = cx.nc
    items, _ = PACK_META[blk]
    for k, rows, cols, off in items:
        if k == wn:
            wt = pool.tile([P, nk, cn], f16, tag=tag, bufs=bufs,
                           name=f"{tag}_{cx.uid()}")
            rpr = rows // R
            for kk in range(nk):
                row0 = (kbase + kk) * P
                rank, rrow = row0 // rpr, row0 % rpr
                apv = cx.wpacks[blk][rank, ds(off + rrow * cols, P * cols)]
                apm = apv.rearrange("(p c) -> p c", c=cols)
                nc.sync.dma_start(wt[:, kk], apm[:, ds(c0, cn)])
            return wt
    raise KeyError(wn)


def split16(cx, pool, src_ap, tag, Tc, rows=P, bufs=1):
    nc = cx.nc
    hi = pool.tile([rows, Tc], f16, tag=f"{tag}h", bufs=bufs,
                   name=f"{tag}h_{cx.uid()}")
    nc.vector.tensor_copy(hi[:], src_ap)
    return hi


def rmsnorm(cx, pool, x_tiles, g_row, T, tag):
    nc = cx.nc
    sq = pool.tile([P, T], f32, tag="nsq", bufs=2, name=f"nsq_{cx.uid()}")
    ssum = cx.psC.tile([1, T], f32, tag="mis1", name=f"nss_{cx.uid()}")
    for ko in range(KD):
        nc.vector.tensor_mul(sq[:], x_tiles[ko][:], x_tiles[ko][:])
        nc.tensor.matmul(ssum[:], cx.ones_col[:], sq[:],
                         start=(ko == 0), stop=(ko == KD - 1))
    rms = pool.tile([1, T], f32, tag="nrm", bufs=1, name=f"nrm_{cx.uid()}")
    nc.vector.tensor_scalar(rms[:], ssum[:], 1.0 / D, EPS, op0=OP.mult, op1=OP.add)
    nc.scalar.activation(rms[:], rms[:], AF.Sqrt)
    rinv = pool.tile([1, T], f32, tag="nri", bufs=1, name=f"nri_{cx.uid()}")
    nc.vector.reciprocal(rinv[:], rms[:])
    bc = cx.psC.tile([P, T], f32, tag="mis2", name=f"nbc_{cx.uid()}")
    nc.tensor.matmul(bc[:], cx.ones_row[:], rinv[:], start=True, stop=True)
    bcs = pool.tile([P, T], f32, tag="nbcs", bufs=1, name=f"nbcs_{cx.uid()}")
    nc.vector.tensor_copy(bcs[:], bc[:])
    out = []
    for ko in range(KD):
        xn = pool.tile([P, T], f32, tag="nxn", bufs=2, name=f"nxn_{cx.uid()}")
        nc.vector.tensor_mul(xn[:], x_tiles[ko][:], bcs[:])
        nc.vector.tensor_tensor(
            xn[:, None, :], xn[:, None, :],
            cx.ln_sb[:, g_row, ko, None, None].to_broadcast([P, 1, T]), OP.mult)
        out.append(split16(cx, pool, xn[:], f"{tag}{ko}", T))
    return out


def linear_res(cx, wt, xin, T, Mtiles, out_cb):
    """out[m] = sum_ko wt[:,ko,m-slice].T @ xin[ko], Mtiles output tiles."""
    nc = cx.nc
    for mg in range(0, Mtiles, 2):
        pts = [cx.psA.tile([P, T], f32, tag=("ps" if mi == 0 else "sc"),
                           name=f"lps{mi}_{cx.uid()}") for mi in range(2)]
        for ko in range(KD):
            for mi in range(2):
                nc.tensor.matmul(pts[mi][:], wt[:, ko, ds((mg + mi) * P, P)],
                                 xin[ko][:], start=(ko == 0), stop=(ko == KD - 1))
        for mi in range(2):
            out_cb(mg + mi, pts[mi])


def llama_block(cx, dram, x_tiles, blk, T):
    nc, tc = cx.nc, cx.tc
    SK = T // P
    tg = f"b{blk}"
    KVL = D * T + T * H * E1

    with tc.tile_pool(name=f"bp{blk}", bufs=1) as bp:
        q_sp = [None] * KD
        kvloc = dram.tile([KVL], f16, tag=f"{tg}kv", name=f"{tg}kv")
        kloc = kvloc[ds(0, D * T)].rearrange("(d t) -> d t", t=T)
        vloc = kvloc[ds(D * T, T * H * E1)].rearrange("(t e) -> t e", e=H * E1)

        with tc.tile_pool(name=f"qk{blk}", bufs=2) as sp:
            xn = rmsnorm(cx, sp, x_tiles, 2 * blk, T, "xn")

            wkt = load_wgroup(cx, sp, blk, 'wk', 0, 0, D, "wqkv")

            def k_cb(mo, pt):
                kh = split16(cx, sp, pt[:], "kk", T, bufs=2)
                nc.sync.dma_start(kloc[ds(mo * P, P)], kh[:])

            linear_res(cx, wkt, xn, T, KD, k_cb)

            wvt = load_wgroup(cx, sp, blk, 'wv', 0, 0, D, "wqkv")
            for tt in range(SK):
                vsb = sp.tile([P, H * E1], f16, tag="vsb", bufs=2,
                              name=f"vsb_{cx.uid()}")
                nc.vector.memset(vsb[:], 1.0)
                for nc2 in range(D // 512):
                    pt = cx.psA.tile([P, 512], f32, tag="ps", name=f"vps_{cx.uid()}")
                    for ko in range(KD):
                        nc.tensor.matmul(pt[:], xn[ko][:, ts(tt, P)],
                                         wvt[:, ko, ds(nc2 * 512, 512)],
                                         start=(ko == 0), stop=(ko == KD - 1))
                    nh = 512 // DH
                    nc.vector.tensor_copy(
                        vsb[:, ds(nc2 * nh * E1, nh * E1)].rearrange(
                            "p (h e) -> p h e", e=E1)[:, :, :DH],
                        pt[:].rearrange("p (h e) -> p h e", e=DH))
                nc.sync.dma_start(vloc[ds(tt * P, P)], vsb[:])

            kvall = dram.tile([R, KVL], f16, tag=f"{tg}kva", name=f"{tg}kva",
                              addr_space="Shared")
            nc.gpsimd.collective_compute(
                "AllGather", OP.bypass, replica_groups=REPL,
                ins=[kvloc[:].opt()], outs=[kvall[:].opt()])

            # Q computed after the gather is kicked off — overlaps AG latency
            wqt = load_wgroup(cx, sp, blk, 'wq', 0, 0, D, "wqkv")

            def q_cb(mo, pt):
                q_sp[mo] = split16(cx, bp, pt[:], f"qs{mo}", T)

            linear_res(cx, wqt, xn, T, KD, q_cb)

        kvall_f = kvall[:].rearrange("g l -> (g l)")

        def kslice(jrel, d0):
            if jrel == 0:   # own K, no AG dependency
                return kvloc[ds(d0 * T, P * T)].rearrange("(p t) -> p t", t=T)
            src = cx.srcs[jrel]
            return kvall_f[ds(src * KVL + d0 * T, P * T)].rearrange(
                "(p t) -> p t", t=T)

        def vslice(jrel, t0):
            if jrel == 0:
                return kvloc[ds(D * T + t0 * H * E1, P * H * E1)].rearrange(
                    "(p e) -> p e", e=H * E1)
            src = cx.srcs[jrel]
            return kvall_f[ds(src * KVL + D * T + t0 * H * E1,
                              P * H * E1)].rearrange("(p e) -> p e", e=H * E1)

        attn_sp = [None] * KD
        with tc.tile_pool(name=f"at{blk}", bufs=2) as sp:
            vbufs = []
            for jrel in range(NRANK):
                vb = sp.tile([P, SK, H * E1], f16, tag=f"vb{jrel}", bufs=1,
                             name=f"vb{jrel}_{cx.uid()}")
                for kk in range(SK):
                    nc.sync.dma_start(vb[:, kk], vslice(jrel, kk * P))
                vbufs.append(vb)
            for hp in range(H // 2):
                kbufs = []
                for jrel in range(NRANK):
                    kb = sp.tile([P, T], f16, tag=f"kb{jrel}", bufs=2,
                                 name=f"kb{jrel}_{cx.uid()}")
                    nc.sync.dma_start(kb[:], kslice(jrel, hp * P))
                    kbufs.append(kb)
                recip = sp.tile([33, T], f32, tag="rc", bufs=2, name=f"rc_{cx.uid()}")
                ovs = []
                for hpar in range(2):
                    h = 2 * hp + hpar
                    qrow = DH * hpar
                    rh = q_sp[hp][ds(qrow, DH)]
                    ov = cx.psB.tile([E1, T], f32, tag="ov",
                                     name=f"ov_{cx.uid()}")
                    total_sk = NRANK * SK
                    isk = 0
                    for jrel in range(NRANK):
                        for kk in range(SK):
                            sc = cx.psA.tile([P, T], f32, tag="sc",
                                             name=f"sc_{cx.uid()}")
                            nc.tensor.matmul(sc[:],
                                             kbufs[jrel][ds(qrow, DH), ts(kk, P)],
                                             rh, start=True, stop=True)
                            ex = sp.tile([P, T], f16, tag="ex", bufs=2,
                                         name=f"ex_{cx.uid()}")
                            if jrel == 0:
                                tmp = sp.tile([P, T], f32, tag="ext", bufs=2,
                                              name=f"ext_{cx.uid()}")
                                nc.scalar.activation(tmp[:], sc[:], AF.Exp, scale=ISQ)
                                nc.gpsimd.affine_select(
                                    ex[:], tmp[:], pattern=[[1, T]],
                                    compare_op=OP.is_ge, fill=0.0,
                                    base=-kk * P, channel_multiplier=-1)
                            else:
                                nc.scalar.activation(ex[:], sc[:], AF.Exp, scale=ISQ,
                                                     bias=cx.ab_sb[:, jrel:jrel + 1])
                            nc.tensor.matmul(ov[:],
                                             vbufs[jrel][:, kk, ds(h * E1, E1)],
                                             ex[:], start=(isk == 0),
                                             stop=(isk == total_sk - 1))
                            isk += 1
                    nc.vector.reciprocal(recip[ds(32 * hpar, 1)], ov[ds(DH, 1)])
                    ovs.append(ov)
                nbc = cx.psC.tile([P, T], f32, tag="mis2", name=f"nb_{cx.uid()}")
                nc.tensor.matmul(nbc[:], cx.sel2[:], recip[:], start=True, stop=True)
                nbs = sp.tile([P, T], f32, tag="nbs", bufs=2, name=f"nbs_{cx.uid()}")
                nc.vector.tensor_copy(nbs[:], nbc[:])
                at_f = sp.tile([P, T], f32, tag="atf", bufs=2, name=f"atf_{cx.uid()}")
                nc.vector.tensor_mul(at_f[ds(0, DH)], ovs[0][ds(0, DH)],
                                     nbs[ds(0, DH)])
                nc.vector.tensor_mul(at_f[ds(DH, DH)], ovs[1][ds(0, DH)],
                                     nbs[ds(DH, DH)])
                attn_sp[hp] = split16(cx, bp, at_f[:], f"as{hp}", T)

        with tc.tile_pool(name=f"op{blk}", bufs=2) as sp:
            wot = load_wgroup(cx, sp, blk, 'wo', 0, 0, D, "wot")

            def o_cb(mo, pt):
                nc.vector.tensor_add(x_tiles[mo][:], x_tiles[mo][:], pt[:])

            linear_res(cx, wot, attn_sp, T, KD, o_cb)

    with tc.tile_pool(name=f"ml{blk}", bufs=2) as sp:
        xn2 = rmsnorm(cx, sp, x_tiles, 2 * blk + 1, T, "xm")
        for g0 in range(0, KF, KGRP):
            wgt = load_wgroup(cx, sp, blk, 'wg', 0, g0 * P, KGRP * P, "wgt")
            wut = load_wgroup(cx, sp, blk, 'wu', 0, g0 * P, KGRP * P, "wut")
            gu_sp = [None] * KGRP
            for f0 in range(0, KGRP, 2):
                gps = [cx.psA.tile([P, T], f32, tag=t_, name=f"g{mi}_{cx.uid()}")
                       for mi, t_ in enumerate(("ps", "sc"))]
                ups = [cx.psB.tile([P, T], f32, tag="ov", name=f"u0_{cx.uid()}"),
                       cx.psC.tile([P, T], f32, tag="mis2", name=f"u1_{cx.uid()}")]
                for ko in range(KD):
                    xh = xn2[ko]
                    for mi in range(2):
                        nc.tensor.matmul(gps[mi][:],
                                         wgt[:, ko, ds((f0 + mi) * P, P)], xh[:],
                                         start=(ko == 0), stop=(ko == KD - 1))
                        nc.tensor.matmul(ups[mi][:],
                                         wut[:, ko, ds((f0 + mi) * P, P)], xh[:],
                                         start=(ko == 0), stop=(ko == KD - 1))
                for mi in range(2):
                    gs = sp.tile([P, T], f32, tag="gss", bufs=2,
                                 name=f"gss_{cx.uid()}")
                    nc.scalar.activation(gs[:], gps[mi][:], AF.Silu)
                    gu_f = sp.tile([P, T], f32, tag="guf", bufs=2,
                                   name=f"guf_{cx.uid()}")
                    nc.vector.tensor_mul(gu_f[:], gs[:], ups[mi][:])
                    gu_sp[f0 + mi] = split16(cx, sp, gu_f[:], f"gu{f0 + mi}", T)
            wdt = load_wgroup(cx, sp, blk, 'wd', g0, 0, D, "wdt", nk=KGRP)
            for mg in range(0, KD, 2):
                pts = [cx.psA.tile([P, T], f32, tag=t_, name=f"d{mi}_{cx.uid()}")
                       for mi, t_ in enumerate(("ps", "sc"))]
                for k2 in range(KGRP):
                    for mi in range(2):
                        nc.tensor.matmul(pts[mi][:],
                                         wdt[:, k2, ds((mg + mi) * P, P)],
                                         gu_sp[k2][:],
                                         start=(k2 == 0), stop=(k2 == KGRP - 1))
                for mi in range(2):
                    nc.vector.tensor_add(x_tiles[mg + mi][:],
                                         x_tiles[mg + mi][:], pts[mi][:])


def dve_matvec(cx, pool, x_tiles, rw_row, T):
    nc = cx.nc
    acc = pool.tile([P, T], f32, tag="mvac", bufs=1, name=f"mvac_{cx.uid()}")
    tmp = pool.tile([P, T], f32, tag="mvtp", bufs=1, name=f"mvtp_{cx.uid()}")
    for ko in range(KD):
        dst = acc if ko == 0 else tmp
        nc.vector.tensor_tensor(
            dst[:, None, :], x_tiles[ko][:, None, :],
            cx.rw_sb[:, rw_row, ko, None, None].to_broadcast([P, 1, T]), OP.mult)
        if ko > 0:
            nc.vector.tensor_add(acc[:], acc[:], tmp[:])
    pt = cx.psC.tile([1, T], f32, tag="mis1", name=f"mv_{cx.uid()}")
    nc.tensor.matmul(pt[:], cx.ones_col[:], acc[:], start=True, stop=True)
    lg = pool.tile([1, T], f32, tag="mvlg", bufs=1, name=f"mvlg_{cx.uid()}")
    nc.vector.tensor_copy(lg[:], pt[:])
    return lg


def topk_mask(cx, pool, lall_flat, Sb, ktarget):
    """Threshold for top-k via 4 levels of 128-way parallel counting.

    Invariant: count(lg > lo) >= k. Final resolution 32/128^4 = 1.2e-7.
    """
    nc = cx.nc
    nb = Sb // P
    lg = pool.tile([P, B, nb], f32, tag="bilg", bufs=1, name=f"bilg_{cx.uid()}")
    for bb in range(B):
        nc.sync.dma_start(lg[:, bb],
                          lall_flat[ds(bb * Sb, Sb)].rearrange("(p c) -> p c", c=nb))
    thrb = pool.tile([P, B], f32, tag="tkthb", bufs=1, name=f"tkthb_{cx.uid()}")
    for bb in range(B):
        # broadcast this batch's logits to all partitions: cbc[p, j] = lg[j]
        lgrow = pool.tile([1, Sb], f32, tag=f"tkrow{bb}", bufs=1,
                          name=f"tkrow_{cx.uid()}")
        nc.sync.dma_start(lgrow[:], lall_flat[None, ds(bb * Sb, Sb)])
        cbc = pool.tile([P, Sb], f32, tag=f"tkcbc{bb}", bufs=1,
                        name=f"tkcbc_{cx.uid()}")
        for chk in range(0, Sb, 512):
            w = min(512, Sb - chk)
            pt = cx.psC.tile([P, 512], f32, tag="mis2", name=f"tkq_{cx.uid()}")
            nc.tensor.matmul(pt[:, :w], cx.ones_row[:], lgrow[:, ds(chk, w)],
                             start=True, stop=True)
            nc.vector.tensor_copy(cbc[:, ds(chk, w)], pt[:, :w])
        loS = pool.tile([1, 1], f32, tag=f"tklo{bb}", bufs=1,
                        name=f"tklo_{cx.uid()}")
        nc.vector.memset(loS[:], -16.0)
        lo_bc = pool.tile([P, 1], f32, tag=f"tklob{bb}", bufs=1,
                          name=f"tklob_{cx.uid()}")
        nc.vector.memset(lo_bc[:], -16.0)
        for lvl in range(4):
            step = 32.0 / (P ** (lvl + 1))
            thr = pool.tile([P, 1], f32, tag=f"tkthr{bb}", bufs=2,
                            name=f"tkthr_{cx.uid()}")
            nc.vector.tensor_scalar(thr[:], cx.iota_f[:], step, step,
                                    op0=OP.mult, op1=OP.add)
            nc.vector.tensor_add(thr[:], thr[:], lo_bc[:])
            cmp = pool.tile([P, Sb], f32, tag=f"tkcmp{bb}", bufs=2,
                            name=f"tkcmp_{cx.uid()}")
            nc.vector.tensor_tensor(cmp[:], cbc[:], thr[:].to_broadcast([P, Sb]),
                                    OP.is_gt)
            red = pool.tile([P, 1], f32, tag=f"tkred{bb}", bufs=2,
                            name=f"tkred_{cx.uid()}")
            nc.vector.tensor_reduce(red[:], cmp[:], axis=mybir.AxisListType.X,
                                    op=OP.add)
            selp = pool.tile([P, 1], f32, tag=f"tksel{bb}", bufs=2,
                             name=f"tksel_{cx.uid()}")
            nc.vector.tensor_scalar(selp[:], red[:], float(ktarget), None,
                                    op0=OP.is_ge)
            cnt = cx.psC.tile([1, 1], f32, tag="mis1", name=f"tkc_{cx.uid()}")
            nc.tensor.matmul(cnt[:], cx.ones_col[:], selp[:], start=True, stop=True)
            stp = pool.tile([1, 1], f32, tag=f"tkst{bb}", bufs=2,
                            name=f"tkst_{cx.uid()}")
            nc.vector.tensor_scalar_mul(stp[:], cnt[:], step)
            nc.vector.tensor_add(loS[:], loS[:], stp[:])
            ptb = cx.psC.tile([P, 1], f32, tag="mis2", name=f"tkb_{cx.uid()}")
            nc.tensor.matmul(ptb[:], cx.ones_row[:], loS[:], start=True, stop=True)
            nc.vector.tensor_copy(lo_bc[:], ptb[:])
        nc.vector.tensor_copy(thrb[:, bb:bb + 1], lo_bc[:])
    mask = pool.tile([P, B, nb], f32, tag="bimk", bufs=1, name=f"bimk_{cx.uid()}")
    nc.vector.tensor_tensor(mask[:], lg[:],
                            thrb[:, :, None].to_broadcast([P, B, nb]), OP.is_gt)
    return mask


def cumsum_pos(cx, pool, dram, mask, Sb, ksel, tag):
    nc = cx.nc
    nb = Sb // P
    a = mask
    s, pp = 1, 0
    while s < nb:
        bt = pool.tile([P, B, nb], f32, tag=f"cs{pp % 2}", bufs=1,
                       name=f"cs_{cx.uid()}")
        nc.vector.tensor_copy(bt[:, :, :s], a[:, :, :s])
        nc.vector.tensor_add(bt[:, :, s:], a[:, :, s:], a[:, :, :nb - s])
        a = bt
        s *= 2
        pp += 1
    tot = pool.tile([P, B], f32, tag="cstt", bufs=1, name=f"cstt_{cx.uid()}")
    nc.vector.tensor_copy(tot[:], a[:, :, nb - 1])
    ppf = cx.psC.tile([P, B], f32, tag="mis2", name=f"csp_{cx.uid()}")
    nc.tensor.matmul(ppf[:], cx.triu[:], tot[:], start=True, stop=True)
    cs = pool.tile([P, B, nb], f32, tag="cscs", bufs=1, name=f"cscs_{cx.uid()}")
    nc.vector.tensor_tensor(cs[:], a[:], ppf[:, :, None].to_broadcast([P, B, nb]),
                            OP.add)
    csd = dram.tile([B, Sb], f32, tag=f"{tag}csd", name=f"{tag}csd")
    nc.sync.dma_start(csd[:].rearrange("b (p c) -> p b c", p=P), cs[:])
    posd = dram.tile([B * ksel, 1], f32, tag=f"{tag}posd", name=f"{tag}posd")
    for bb in range(B):
        csrow = pool.tile([1, Sb], f32, tag="cscr", bufs=1, name=f"cscr_{cx.uid()}")
        nc.sync.dma_start(csrow[:], csd[bb, None, :])
        cbc = pool.tile([P, Sb], f32, tag="cscb", bufs=1, name=f"cscb_{cx.uid()}")
        for ch in range(0, Sb, 512):
            w = min(512, Sb - ch)
            pt = cx.psC.tile([P, 512], f32, tag="mis2", name=f"csq_{cx.uid()}")
            nc.tensor.matmul(pt[:, :w], cx.ones_row[:], csrow[:, ds(ch, w)],
                             start=True, stop=True)
            nc.vector.tensor_copy(cbc[:, ds(ch, w)], pt[:, :w])
        for rt in range(ksel // P):
            rcol = pool.tile([P, 1], f32, tag="csrc", bufs=2, name=f"csrc_{cx.uid()}")
            nc.vector.tensor_scalar_add(rcol[:], cx.iota_f[:], float(rt * P))
            cmp = pool.tile([P, Sb], f32, tag="cscm", bufs=2, name=f"cscm_{cx.uid()}")
            nc.vector.tensor_tensor(cmp[:], cbc[:], rcol[:].to_broadcast([P, Sb]),
                                    OP.is_le)
            red = pool.tile([P, 1], f32, tag="csrd", bufs=2, name=f"csrd_{cx.uid()}")
            nc.vector.tensor_reduce(red[:], cmp[:], axis=mybir.AxisListType.X,
                                    op=OP.add)
            nc.sync.dma_start(posd[ds(bb * ksel + rt * P, P)], red[:])
    return posd


def to_tok_dram(cx, pool, dtile, x_tiles, T):
    nc = cx.nc
    for tt in range(T // P):
        asm = pool.tile([P, D], f32, tag="tkas", bufs=2, name=f"tkas_{cx.uid()}")
        for ko in range(KD):
            tr = cx.psC.tile([P, P], f32, tag="mis2", name=f"tktr_{cx.uid()}")
            nc.tensor.transpose(tr[:], x_tiles[ko][:, ts(tt, P)], cx.ident[:])
            nc.vector.tensor_copy(asm[:, ts(ko, P)], tr[:])
        nc.sync.dma_start(dtile[ds(tt * P, P)], asm[:])


def gather_sel(cx, pool, res, src_flat, posd, T, boff_col, rtag):
    nc = cx.nc
    myoff = cx.pid * T
    xt = [res.tile([P, T], f32, tag=f"{rtag}{ko}", name=f"{rtag}{ko}")
          for ko in range(KD)]
    for u in range(T // P):
        pv = pool.tile([P, 1], f32, tag="gspv", bufs=2, name=f"gspv_{cx.uid()}")
        nc.sync.dma_start(pv[:], posd[ds(myoff + u * P, P)])
        nc.vector.tensor_scalar(pv[:], pv[:], boff_col, None, op0=OP.add)
        pi = pool.tile([P, 1], i32, tag="gspi", bufs=2, name=f"gspi_{cx.uid()}")
        nc.vector.tensor_copy(pi[:], pv[:])
        g = pool.tile([P, D], f32, tag="gsg", bufs=2, name=f"gsg_{cx.uid()}")
        nc.gpsimd.indirect_dma_start(
            out=g[:], out_offset=None, in_=src_flat,
            in_offset=bass.IndirectOffsetOnAxis(ap=pi[:, :1], axis=0))
        for ko in range(KD):
            tr = cx.psC.tile([P, P], f32, tag="mis2", name=f"gstr_{cx.uid()}")
            nc.tensor.transpose(tr[:], g[:, ts(ko, P)], cx.ident[:])
            nc.vector.tensor_copy(xt[ko][:, ts(u, P)], tr[:])
    return xt


def topw_bcast(cx, pool, sel_in, rw_row, T):
    nc = cx.nc
    lgs = dve_matvec(cx, pool, sel_in, rw_row, T)
    tw = pool.tile([1, T], f32, tag="twr", bufs=1, name=f"twr_{cx.uid()}")
    nc.scalar.activation(tw[:], lgs[:], AF.Sigmoid)
    nc.vector.tensor_scalar_mul(tw[:], tw[:], ALPHA)
    pt = cx.psC.tile([P, T], f32, tag="mis2", name=f"twp_{cx.uid()}")
    nc.tensor.matmul(pt[:], cx.ones_row[:], tw[:], start=True, stop=True)
    twb = pool.tile([P, T], f32, tag="twb", bufs=1, name=f"twb_{cx.uid()}")
    nc.vector.tensor_copy(twb[:], pt[:])
    return twb


def build_program(stages=4, dbg=False):
    nc = bacc.Bacc("TRN2", target_bir_lowering=False)
    cx = CX()
    cx.nc = nc
    cx._u = 0

    def uid():
        cx._u += 1
        return cx._u
    cx.uid = uid

    innames = ["h0T", "ln", "rw", "abias", "fvec", "sel2c"]
    h0T = nc.declare_dram_parameter("h0T", [D, T0], f32, isOutput=False)
    lnp = nc.declare_dram_parameter("ln", [13, D], f32, isOutput=False)
    rwp = nc.declare_dram_parameter("rw", [2, D], f32, isOutput=False)
    abp = nc.declare_dram_parameter("abias", [NRANK, P], f32, isOutput=False)
    fvp = nc.declare_dram_parameter("fvec", [P, 4], f32, isOutput=False)
    s2p = nc.declare_dram_parameter("sel2c", [33, P], f32, isOutput=False)
    nblk = 6 if stages >= 3 else (3 if stages >= 2 else 1)
    wparams = {}
    for blk in range(nblk):
        items, shard = PACK_META[blk]
        wparams[blk] = nc.declare_dram_parameter(f"wpack{blk}", [R, shard], f16,
                                                 isOutput=False)
        innames.append(f"wpack{blk}")
    out = embT = None
    if stages >= 4:
        embT = nc.declare_dram_parameter("embT", [D, VS], f16, isOutput=False)
        out = nc.declare_dram_parameter("out", [B * S, VS], f32, isOutput=True)
        innames.append("embT")
    dbg_o = {}

    def dbg_out(nm, shp):
        dbg_o[nm] = nc.declare_dram_parameter(nm, shp, f32, isOutput=True)
        return dbg_o[nm]

    with tile.TileContext(nc) as tc:
        cx.tc = tc
        with (
            tc.tile_pool(name="const", bufs=1) as cst,
            tc.tile_pool(name="res", bufs=1) as res,
            tc.tile_pool(name="psA", bufs=2, space="PSUM") as psA,
            tc.tile_pool(name="psB", bufs=2, space="PSUM") as psB,
            tc.tile_pool(name="psC", bufs=1, space="PSUM") as psC,
            tc.tile_pool(name="dram", bufs=1, space="DRAM") as dram,
        ):
            cx.psA, cx.psB, cx.psC = psA, psB, psC

            cx.ones_col = cst.tile([P, 1], f32, name="ones_col")
            nc.vector.memset(cx.ones_col[:], 1.0)
            cx.ones_row = cst.tile([1, P], f32, name="ones_row")
            nc.vector.memset(cx.ones_row[:], 1.0)
            cx.sel2 = cst.tile([33, P], f32, name="sel2")
            nc.sync.dma_start(cx.sel2[:], s2p.ap())
            cx.ident = cst.tile([P, P], f32, name="ident")
            make_identity(nc, cx.ident[:])
            onespp = cst.tile([P, P], f32, name="onespp")
            nc.vector.memset(onespp[:], 1.0)
            cx.triu = cst.tile([P, P], f32, name="triu")
            nc.gpsimd.affine_select(cx.triu[:], onespp[:], pattern=[[1, P]],
                                    compare_op=OP.is_ge, fill=0.0, base=-1,
                                    channel_multiplier=-1)
            iota_i = cst.tile([P, 1], i32, name="iota_i")
            nc.gpsimd.iota(iota_i[:], pattern=[[0, 1]], base=0, channel_multiplier=1)
            cx.iota_f = cst.tile([P, 1], f32, name="iota_f")
            nc.vector.tensor_copy(cx.iota_f[:], iota_i[:])
            cx.ln_sb = cst.tile([P, 13, KD], f32, name="ln_sb")
            nc.sync.dma_start(cx.ln_sb[:],
                              lnp.ap().rearrange("r (ko p) -> p r ko", p=P))
            cx.rw_sb = cst.tile([P, 2, KD], f32, name="rw_sb")
            nc.sync.dma_start(cx.rw_sb[:],
                              rwp.ap().rearrange("r (ko p) -> p r ko", p=P))
            cx.ab_sb = cst.tile([P, NRANK], f32, name="ab_sb")
            nc.sync.dma_start(cx.ab_sb[:], abp.ap().rearrange("j p -> p j"))
            cx.fv_sb = cst.tile([P, 4], f32, name="fv_sb")
            nc.sync.dma_start(cx.fv_sb[:], fvp.ap())

            pid = nc.sync.partition_id()
            cx.pid = pid
            qreg = pid % NRANK
            base = pid - qreg
            cx.srcs = [smax(pid - j, base) for j in range(NRANK)]

            cx.wpacks = {}
            for blk in range(nblk):
                cx.wpacks[blk] = wparams[blk].ap()

            # ---- stage 1: block 0 + recursion-0 routing
            with tc.tile_pool(name="st1", bufs=1) as st1:
                x = [st1.tile([P, T0], f32, tag=f"xa{ko}", name=f"xa{ko}")
                     for ko in range(KD)]
                h0ap = h0T.ap().rearrange("(ko p) t -> p ko t", p=P)
                for ko in range(KD):
                    nc.sync.dma_start(x[ko][:], h0ap[:, ko])
                llama_block(cx, dram, x, 0, T0)

                with tc.tile_pool(name="rt0", bufs=2) as rp:
                    lg0 = dve_matvec(cx, rp, x, 0, T0)
                    lloc = dram.tile([1, T0], f32, tag="lloc0", name="lloc0")
                    nc.sync.dma_start(lloc[:], lg0[:])
                    lall = dram.tile([R, 1, T0], f32, tag="lall0", name="lall0",
                                     addr_space="Shared")
                    nc.gpsimd.collective_compute(
                        "AllGather", OP.bypass, replica_groups=REPL,
                        ins=[lloc[:].opt()], outs=[lall[:].opt()])
                    htl = dram.tile([T0, D], f32, tag="htl", name="htl")
                    to_tok_dram(cx, rp, htl, x, T0)
                    hta = dram.tile([R, T0, D], f32, tag="hta", name="hta",
                                    addr_space="Shared")
                    nc.gpsimd.collective_compute(
                        "AllGather", OP.bypass, replica_groups=REPL,
                        ins=[htl[:].opt()], outs=[hta[:].opt()])
                    cx.hta_r = hta[:].rearrange("r t d -> (r t) d")

                    mask0 = topk_mask(cx, rp,
                                        lall[:].rearrange("r o t -> (r o t)"),
                                        S, S // 2)
                    posd0 = cumsum_pos(cx, rp, dram, mask0, S, S // 2, "c0")
                    seli = gather_sel(cx, rp, res, cx.hta_r, posd0, T1,
                                      cx.fv_sb[:, 0:1], "sli")
                    if dbg:
                        d1 = dbg_out("dbg_h0b", [T0, D])
                        nc.sync.dma_start(d1.ap(), htl[:])
                        d2 = dbg_out("dbg_lg", [1, T0])
                        nc.sync.dma_start(d2.ap(), lloc[:])
                        d3 = dbg_out("dbg_pos", [B * S // 2, 1])
                        nc.sync.dma_start(d3.ap(), posd0[:])
                        d4 = dbg_out("dbg_selT", [D, T1])
                        d4r = d4.ap().rearrange("(ko p) t -> p ko t", p=P)
                        for ko in range(KD):
                            nc.sync.dma_start(d4r[:, ko], seli[ko][:])

            if stages >= 2:
                with tc.tile_pool(name="st2", bufs=1) as st2:
                    sel = [st2.tile([P, T1], f32, tag=f"sl{ko}", name=f"sl{ko}")
                           for ko in range(KD)]
                    for ko in range(KD):
                        nc.vector.tensor_copy(sel[ko][:], seli[ko][:])
                    llama_block(cx, dram, sel, 1, T1)
                    llama_block(cx, dram, sel, 2, T1)
                    with tc.tile_pool(name="rt1", bufs=2) as rp:
                        twb0 = topw_bcast(cx, rp, seli, 0, T1)
                        x1 = [res.tile([P, T1], f32, tag=f"x1{ko}", name=f"x1{ko}")
                              for ko in range(KD)]
                        for ko in range(KD):
                            nc.vector.tensor_mul(x1[ko][:], sel[ko][:], twb0[:])
                            nc.vector.tensor_add(x1[ko][:], x1[ko][:], seli[ko][:])
                        lg1 = dve_matvec(cx, rp, x1, 1, T1)
                        lloc1 = dram.tile([1, T1], f32, tag="lloc1", name="lloc1")
                        nc.sync.dma_start(lloc1[:], lg1[:])
                        lall1 = dram.tile([R, 1, T1], f32, tag="lall1",
                                          name="lall1", addr_space="Shared")
                        nc.gpsimd.collective_compute(
                            "AllGather", OP.bypass, replica_groups=REPL,
                            ins=[lloc1[:].opt()], outs=[lall1[:].opt()])
                        x1l = dram.tile([T1, D], f32, tag="x1l", name="x1l")
                        to_tok_dram(cx, rp, x1l, x1, T1)
                        x1a = dram.tile([R, T1, D], f32, tag="x1a", name="x1a",
                                        addr_space="Shared")
                        nc.gpsimd.collective_compute(
                            "AllGather", OP.bypass, replica_groups=REPL,
                            ins=[x1l[:].opt()], outs=[x1a[:].opt()])
                        cx.x1a_r = x1a[:].rearrange("r t d -> (r t) d")

                        mask1 = topk_mask(cx, rp,
                                            lall1[:].rearrange("r o t -> (r o t)"),
                                            S // 2, S // 4)
                        posd1 = cumsum_pos(cx, rp, dram, mask1, S // 2, S // 4, "c1")
                        sl1i = gather_sel(cx, rp, res, cx.x1a_r, posd1, T2,
                                          cx.fv_sb[:, 1:2], "s1i")
                        if dbg:
                            d5 = dbg_out("dbg_x1", [T1, D])
                            nc.sync.dma_start(d5.ap(), x1l[:])
                            d6 = dbg_out("dbg_pos1", [B * S // 4, 1])
                            nc.sync.dma_start(d6.ap(), posd1[:])

            if stages >= 3:
                with tc.tile_pool(name="st3", bufs=1) as st3:
                    sl1 = [st3.tile([P, T2], f32, tag=f"sm{ko}", name=f"sm{ko}")
                           for ko in range(KD)]
                    for ko in range(KD):
                        nc.vector.tensor_copy(sl1[ko][:], sl1i[ko][:])
                    llama_block(cx, dram, sl1, 3, T2)
                    llama_block(cx, dram, sl1, 4, T2)
                    with tc.tile_pool(name="rt2", bufs=2) as rp:
                        twb1 = topw_bcast(cx, rp, sl1i, 1, T2)
                        z = [st3.tile([P, T2], f32, tag=f"zz{ko}", name=f"zz{ko}")
                             for ko in range(KD)]
                        for ko in range(KD):
                            nc.vector.tensor_mul(z[ko][:], sl1[ko][:], twb1[:])
                            nc.vector.tensor_add(z[ko][:], z[ko][:], sl1i[ko][:])
                        zl = dram.tile([T2, D], f32, tag="zl", name="zl")
                        to_tok_dram(cx, rp, zl, z, T2)
                        za = dram.tile([R, T2, D], f32, tag="za", name="za",
                                       addr_space="Shared")
                        nc.gpsimd.collective_compute(
                            "AllGather", OP.bypass, replica_groups=REPL,
                            ins=[zl[:].opt()], outs=[za[:].opt()])
                        za_r = za[:].rearrange("r t d -> (r t) d")

                        h2loc = dram.tile([R * T0, D], f32, tag="h2loc",
                                          name="h2loc")
                        nc.sync.dma_start(h2loc[:], cx.hta_r)
                        cx.h2_r = h2loc[:]

                        for ch in range(B * S // 2 // P):
                            bb = ch // (S // 2 // P)
                            ssb = rp.tile([P, D], f32, tag="scx", bufs=2,
                                          name=f"scx_{cx.uid()}")
                            nc.sync.dma_start(ssb[:], cx.x1a_r[ds(ch * P, P)])
                            pv = rp.tile([P, 1], f32, tag="scp", bufs=2,
                                         name=f"scp_{cx.uid()}")
                            nc.sync.dma_start(pv[:], posd0[ds(ch * P, P)])
                            nc.vector.tensor_scalar_add(pv[:], pv[:], float(bb * S))
                            pi = rp.tile([P, 1], i32, tag="sci", bufs=2,
                                         name=f"sci_{cx.uid()}")
                            nc.vector.tensor_copy(pi[:], pv[:])
                            nc.gpsimd.indirect_dma_start(
                                out=cx.h2_r, out_offset=bass.IndirectOffsetOnAxis(
                                    ap=pi[:, :1], axis=0),
                                in_=ssb[:], in_offset=None)
                        for ch in range(B * S // 4 // P):
                            bb = ch // (S // 4 // P)
                            ssb = rp.tile([P, D], f32, tag="scz", bufs=2,
                                          name=f"scz_{cx.uid()}")
                            nc.sync.dma_start(ssb[:], za_r[ds(ch * P, P)])
                            p1 = rp.tile([P, 1], f32, tag="sc1", bufs=2,
                                         name=f"sc1_{cx.uid()}")
                            nc.sync.dma_start(p1[:], posd1[ds(ch * P, P)])
                            nc.vector.tensor_scalar_add(p1[:], p1[:],
                                                        float(bb * (S // 2)))
                            p1i = rp.tile([P, 1], i32, tag="sc2", bufs=2,
                                          name=f"sc2_{cx.uid()}")
                            nc.vector.tensor_copy(p1i[:], p1[:])
                            p0 = rp.tile([P, 1], f32, tag="sc3", bufs=2,
                                         name=f"sc3_{cx.uid()}")
                            nc.gpsimd.indirect_dma_start(
                                out=p0[:], out_offset=None, in_=posd0[:],
                                in_offset=bass.IndirectOffsetOnAxis(
                                    ap=p1i[:, :1], axis=0))
                            nc.vector.tensor_scalar_add(p0[:], p0[:], float(bb * S))
                            p0i = rp.tile([P, 1], i32, tag="sc4", bufs=2,
                                          name=f"sc4_{cx.uid()}")
                            nc.vector.tensor_copy(p0i[:], p0[:])
                            nc.gpsimd.indirect_dma_start(
                                out=cx.h2_r, out_offset=bass.IndirectOffsetOnAxis(
                                    ap=p0i[:, :1], axis=0),
                                in_=ssb[:], in_offset=None)
                        if dbg:
                            d7 = dbg_out("dbg_h2", [T0, D])
                            nc.sync.dma_start(d7.ap(), cx.h2_r[ds(cx.pid * T0, T0)])

            if stages >= 4:
                with tc.tile_pool(name="st4", bufs=1) as st4:
                    x5 = [st4.tile([P, T0], f32, tag=f"x5{ko}", name=f"x5{ko}")
                          for ko in range(KD)]
                    with tc.tile_pool(name="ld5", bufs=2) as rp:
                        for tt in range(T0 // P):
                            tkb = rp.tile([P, D], f32, tag="h2t", bufs=2,
                                          name=f"h2t_{cx.uid()}")
                            nc.sync.dma_start(tkb[:],
                                              cx.h2_r[ds(cx.pid * T0 + tt * P, P)])
                            for ko in range(KD):
                                tr = cx.psC.tile([P, P], f32, tag="mis2",
                                                 name=f"h2r_{cx.uid()}")
                                nc.tensor.transpose(tr[:], tkb[:, ts(ko, P)],
                                                    cx.ident[:])
                                nc.vector.tensor_copy(x5[ko][:, ts(tt, P)], tr[:])
                    llama_block(cx, dram, x5, 5, T0)
                    hfl = dram.tile([P, KD, T0], f16, tag="hfl", name="hfl")
                    with tc.tile_pool(name="fn5", bufs=2) as rp:
                        hfn = rmsnorm(cx, rp, x5, 12, T0, "hf")
                        for ko in range(KD):
                            nc.sync.dma_start(hfl[:, ko], hfn[ko][:])
                    hfa = dram.tile([R, P, KD, T0], f16, tag="hfa", name="hfa",
                                    addr_space="Shared")
                    nc.gpsimd.collective_compute(
                        "AllGather", OP.bypass, replica_groups=REPL,
                        ins=[hfl[:].opt()], outs=[hfa[:].opt()])
                with tc.tile_pool(name="hd", bufs=1) as hd:
                    NCH = 4
                    CH = VS // NCH  # 1000
                    for ch in range(NCH):
                        et = hd.tile([P, KD, CH], f16, tag="et", bufs=2,
                                     name=f"et_{cx.uid()}")
                        for ko in range(KD):
                            nc.sync.dma_start(
                                et[:, ko],
                                embT.ap()[ds(ko * P, P), ds(ch * CH, CH)])
                        for rr in range(R):
                            hl = hd.tile([P, KD, T0], f16, tag="hl", bufs=2,
                                         name=f"hl_{cx.uid()}")
                            nc.sync.dma_start(hl[:], hfa[rr])
                            for tt in range(T0 // P):
                                for hf2 in range(2):
                                    pt = cx.psA.tile([P, 500], f32, tag="ps",
                                                     name=f"hd_{cx.uid()}")
                                    for ko in range(KD):
                                        nc.tensor.matmul(
                                            pt[:], hl[:, ko, ts(tt, P)],
                                            et[:, ko, ds(hf2 * 500, 500)],
                                            start=(ko == 0), stop=(ko == KD - 1))
                                    ot = hd.tile([P, 500], f32, tag="hot", bufs=3,
                                                 name=f"hot_{cx.uid()}")
                                    nc.vector.tensor_copy(ot[:], pt[:])
                                    nc.sync.dma_start(
                                        out.ap()[ds(rr * T0 + tt * P, P),
                                                 ds(ch * CH + hf2 * 500, 500)],
                                        ot[:])
    nc.finalize()
    return nc, innames, list(dbg_o)


# ----------------------------------------------------------------------- host

_CACHE = {}


def _prepare_inmaps(inputs, stages):
    input_ids = np.asarray(inputs['input_ids'])
    embed = np.asarray(inputs['embed'], dtype=np.float32)
    pos_emb = np.asarray(inputs['pos_emb'], dtype=np.float32)
    h0 = embed[input_ids] + pos_emb[None, :, :]
    ln = np.empty((13, D), np.float32)
    for i in range(6):
        ln[2 * i] = inputs['ln1'][i]
        ln[2 * i + 1] = inputs['ln2'][i]
    ln[12] = inputs['final_norm']
    rw = np.asarray(inputs['router_w'], dtype=np.float32)

    nblk = 6 if stages >= 3 else (3 if stages >= 2 else 1)
    packs = {}
    for blk in range(nblk):
        items, shard = PACK_META[blk]
        pk = np.empty((R, shard), np.float16)
        for key, rows, cols, off in items:
            W = np.asarray(inputs[REFNAMES[key]][blk], dtype=np.float32)
            rpr = rows // R
            n = rpr * cols
            Wm = W.astype(np.float16).reshape(R, n)
            pk[:, off:off + n] = Wm
        packs[blk] = pk

    embT16 = None
    in_maps = []
    for c in range(R):
        b, q = c // NRANK, c % NRANK
        m = {}
        sl = h0[b, q * T0:(q + 1) * T0]
        m['h0T'] = np.ascontiguousarray(sl.T)
        m['ln'] = ln
        m['rw'] = rw
        ab = np.zeros((NRANK, P), np.float32)
        for j in range(NRANK):
            if j > q:
                ab[j] = NEG
        m['abias'] = ab
        m['fvec'] = np.tile(np.array([[b * S, b * (S // 2), 0, 0]], np.float32),
                            (P, 1))
        s2 = np.zeros((33, P), np.float32)
        s2[0, :DH] = 1.0
        s2[32, DH:] = 1.0
        m['sel2c'] = s2
        for blk in range(nblk):
            m[f'wpack{blk}'] = packs[blk]
        if stages >= 4:
            m['embT'] = np.ascontiguousarray(
                embed[c * VS:(c + 1) * VS].T.astype(np.float16))
        in_maps.append(m)
    return in_maps


def run(inputs, stages=4, dbg=False, trace=False):
    key = (stages, dbg)
    if key not in _CACHE:
        _CACHE[key] = build_program(stages, dbg)
    nc, innames, dbgnames = _CACHE[key]
    in_maps = _prepare_inmaps(inputs, stages)
    return run_bass_kernel_spmd(nc, in_maps, core_ids=list(range(R)), trace=trace)


def kernel(**inputs):
    res = run(inputs, stages=4, dbg=False, trace=False)
    parts = [res.results[c]['out'] for c in range(R)]
    full = np.concatenate(parts, axis=1)
    return full.reshape(B, S, V).astype(np.float32)
